# revision 1
# baseline (speedup 1.0000x reference)
"""Trainium2 Bass kernel for nn_NNModel2 (2x NNConv GNN + pooled MLP readout).

Self-contained: accepts FULL inputs, shards edges across 8 NeuronCores
(edge-parallel by dst owner), returns the FULL [256, 1] output.

v2 design:
  - All gathers/transposes/broadcasts of *input-derived* data are done on the
    HOST and fed as per-core tensors (bf16): xsrcT, bcp (pair-broadcast attr),
    scatter one-hot matrices, permuted edge-MLP weights.
  - conv layer z-trick: z[e,(k,i)] = attr[e,k]*x[src,i]; msg = z @ W' done as
    PSUM-accumulated matmuls over 128-row (k,i) blocks. attr broadcast uses
    PAIR tiles (k0 on partitions 0:64, k1 on 64:128); conv2 covers full i-range
    with a partition-rotated copy of h1srcT (s=1 blocks).
  - h1 exchange via AllToAll of per-edge-needed rows (deduped per (src-owner,
    dst-owner) pair) instead of AllGather: ~0.7MB vs 2MB collective payload.
  - Tail: z1 partials computed locally, ReduceScatter over graphs, local
    readout of 32 graphs/core, AllGather of [256,1] result.
"""

import sys

sys.path.insert(0, "/opt/trn_rl_repo")

import numpy as np
import ml_dtypes

from concourse import bacc, bass, mybir
import concourse.tile as tile
from concourse import bass_utils

P = 128
NCORES = 8
N_NODES = 4096
N_EDGES = 8192
N_GRAPHS = 256
DN = 64
DE = 32
H = 256
NSH = N_NODES // NCORES  # 512
NT = NSH // P  # 4
GT = N_GRAPHS // P  # 2

F32 = mybir.dt.float32
BF16 = mybir.dt.bfloat16
I16 = mybir.dt.int16
AF = mybir.ActivationFunctionType
ALU = mybir.AluOpType
BF = ml_dtypes.bfloat16

_cache = {}
_PREP = {}


def _wrap_idx(idx, n):
    idx = np.asarray(idx, dtype=np.int16)
    assert idx.shape == (n,) and n % 16 == 0
    return np.tile(idx.reshape(n // 16, 16).T, (8, 1)).copy()


def _build(e_pad, S, sc_blocks, zb=(False, False, False), upto="full"):
    ET = e_pad // P
    SBT = S // P  # send-buffer tiles
    nc = bacc.Bacc(num_devices=NCORES)

    # ---- per-core inputs (host-prepped)
    xsrc2 = nc.dram_tensor("xsrc2", [P, 2, e_pad], BF16, kind="ExternalInput")
    bcq = nc.dram_tensor("bcq", [P, 8, e_pad], BF16, kind="ExternalInput")
    scm = nc.dram_tensor("scm", [P, len(sc_blocks) * P], BF16, kind="ExternalInput")
    scp = nc.dram_tensor("scp", [P, NT * GT * P], BF16, kind="ExternalInput")
    sel = nc.dram_tensor("sel", [P, (S // P) * NT * P], BF16, kind="ExternalInput")
    xshT = nc.dram_tensor("xshT", [DN + 1, NSH], BF16, kind="ExternalInput")
    h1src_w = nc.dram_tensor("h1src_w", [P, e_pad // 16], I16, kind="ExternalInput")
    identb = nc.dram_tensor("identb", [P, P], BF16, kind="ExternalInput")
    # ---- shared weights (host-permuted, bf16)
    w1p = nc.dram_tensor("w1p", [P, 16, H], BF16, kind="ExternalInput")
    w2p = nc.dram_tensor("w2p", [P, 64, H], BF16, kind="ExternalInput")
    b1p = nc.dram_tensor("b1p", [DN, H], BF16, kind="ExternalInput")
    b2p = nc.dram_tensor("b2p", [P, 2, H], BF16, kind="ExternalInput")
    r1wb = nc.dram_tensor("r1wb", [DN + 1, H], BF16, kind="ExternalInput")
    r2wb = nc.dram_tensor("r2wb", [P, 2, H], BF16, kind="ExternalInput")
    b2sbb = nc.dram_tensor("b2sbb", [1, H], BF16, kind="ExternalInput")
    l1wb = nc.dram_tensor("l1wb", [P, 2, H // 2], BF16, kind="ExternalInput")
    l1brow = nc.dram_tensor("l1brow", [1, H // 2], BF16, kind="ExternalInput")
    l2wrep = nc.dram_tensor("l2wrep", [N_GRAPHS // NCORES, H // 2], F32, kind="ExternalInput")
    l2brep = nc.dram_tensor("l2brep", [N_GRAPHS // NCORES, 1], F32, kind="ExternalInput")
    out = nc.dram_tensor("out", [N_GRAPHS, 1], F32, kind="ExternalOutput")

    def dbg_out(name, shape):
        return nc.dram_tensor(name, shape, F32, kind="ExternalOutput")

    zb1, zb2, zl1 = zb
    rg = [list(range(NCORES))]
    NSC = len(sc_blocks)
    GSH = N_GRAPHS // NCORES  # 32 graphs per core in the tail

    # first bank-touch bookkeeping for agg scatter (bank = n // 2)
    first_touch = {}
    for bi, (e, n) in enumerate(sc_blocks):
        first_touch.setdefault(n // 2, ("sc", bi))
    for n in range(NT):
        first_touch.setdefault(n // 2, ("root", n))

    with tile.TileContext(nc, num_cores=NCORES) as tc:
        with (
            tc.tile_pool(name="const", bufs=1) as cp,
            tc.tile_pool(name="work", bufs=3) as wp,
            tc.tile_pool(name="dram", bufs=1, space="DRAM") as dr,
        ):
            # ======== stage A: loads (SP queue), conv1-critical first.
            # Same-queue DMA transfers start in issue order, so priority ==
            # issue order here.
            bcq_sb = cp.tile([P, 8, e_pad], BF16)
            nc.sync.dma_start(out=bcq_sb[:, 0:2, :], in_=bcq[:, 0:2, :])
            xsrc2_sb = cp.tile([P, 2, e_pad], BF16)
            nc.sync.dma_start(out=xsrc2_sb[:, 0:1, :], in_=xsrc2[:, 0:1, :])
            w1p_sb = cp.tile([P, 16, H], BF16)
            nc.sync.dma_start(out=w1p_sb[:, 0:4, :], in_=w1p[:, 0:4, :])
            nc.sync.dma_start(out=xsrc2_sb[:, 1:2, :], in_=xsrc2[:, 1:2, :])
            b1p_sb = cp.tile([DN, H], BF16)
            nc.sync.dma_start(out=b1p_sb[:], in_=b1p[:])
            for c in range(1, 4):
                nc.sync.dma_start(
                    out=bcq_sb[:, 2 * c : 2 * c + 2, :], in_=bcq[:, 2 * c : 2 * c + 2, :]
                )
                if c == 1:
                    nc.sync.dma_start(out=w1p_sb[:, 4:8, :], in_=w1p[:, 4:8, :])
                if c == 2:
                    nc.sync.dma_start(out=w1p_sb[:, 8:16, :], in_=w1p[:, 8:16, :])
            scm_sb = cp.tile([P, NSC * P], BF16)
            nc.sync.dma_start(out=scm_sb[:], in_=scm[:])
            xshT_sb = cp.tile([DN + 1, NSH], BF16)
            nc.sync.dma_start(out=xshT_sb[:], in_=xshT[:])
            r1wb_sb = cp.tile([DN + 1, H], BF16)
            nc.sync.dma_start(out=r1wb_sb[:], in_=r1wb[:])
            sel_sb = cp.tile([P, (S // P) * NT * P], BF16)
            nc.sync.dma_start(out=sel_sb[:], in_=sel[:])
            h1src_sb = cp.tile([P, e_pad // 16], I16)
            nc.sync.dma_start(out=h1src_sb[:], in_=h1src_w[:])
            ident_sb = cp.tile([P, P], BF16)
            nc.sync.dma_start(out=ident_sb[:], in_=identb[:])
            # conv2/tail loads last (small ones first, then the big w2p)
            a2a_in = dr.tile([S, H], BF16)
            b2p_sb = cp.tile([P, 2, H], BF16)
            nc.sync.dma_start(out=b2p_sb[:], in_=b2p[:])
            r2wb_sb = cp.tile([P, 2, H], BF16)
            nc.sync.dma_start(out=r2wb_sb[:], in_=r2wb[:])
            b2sbb_sb = cp.tile([1, H], BF16)
            nc.sync.dma_start(out=b2sbb_sb[:], in_=b2sbb[:])
            scp_sb = cp.tile([P, NT * GT * P], BF16)
            nc.sync.dma_start(out=scp_sb[:], in_=scp[:])
            l1wb_sb = cp.tile([P, 2, H // 2], BF16)
            nc.sync.dma_start(out=l1wb_sb[:], in_=l1wb[:])
            l1brow_sb = cp.tile([1, H // 2], BF16)
            nc.sync.dma_start(out=l1brow_sb[:], in_=l1brow[:])
            l2w_sb = cp.tile([GSH, H // 2], F32)
            nc.sync.dma_start(out=l2w_sb[:], in_=l2wrep[:])
            l2b_sb = cp.tile([GSH, 1], F32)
            nc.sync.dma_start(out=l2b_sb[:], in_=l2brep[:])
            w2p_sb = cp.tile([P, 64, H], BF16)
            for c in range(4):
                nc.sync.dma_start(
                    out=w2p_sb[:, 16 * c : 16 * c + 16, :],
                    in_=w2p[:, 16 * c : 16 * c + 16, :],
                )

            with tc.tile_pool(name="psA", bufs=1, space="PSUM") as psA:
                # ======== conv1
                msg_ps = [
                    psA.tile([P, 2 * H], F32, space="PSUM", tag=f"msg{j}", name=f"msg1_{j}")
                    for j in range((ET + 1) // 2)
                ]

                def m1(e):
                    return msg_ps[e // 2][:, (e % 2) * H : (e % 2) * H + H]

                msbs = []

                zts1 = []
                for t in range(16):
                    q1, s1 = t // 2, t % 2
                    zt = wp.tile([P, e_pad], BF16, tag=f"zt1_{t}", name=f"zt1_{t}", bufs=1)
                    nc.vector.tensor_tensor(
                        out=zt[:], in0=xsrc2_sb[:, s1, :], in1=bcq_sb[:, q1, :],
                        op=ALU.mult,
                    )
                    zts1.append(zt)
                # e-major accumulation: each msg bank closes early so its
                # PSUM->SBUF copy overlaps the remaining matmuls
                for e in range(ET):
                    if not zb1:
                        nc.tensor.matmul(
                            m1(e), lhsT=xsrc2_sb[0:DN, 0, P * e : P * (e + 1)],
                            rhs=b1p_sb[:], start=(e % 2 == 0), stop=False,
                            skip_group_check=True,
                        )
                    for t in range(16):
                        nc.tensor.matmul(
                            m1(e), lhsT=zts1[t][:, P * e : P * (e + 1)],
                            rhs=w1p_sb[:, t, :],
                            start=(zb1 and t == 0 and e % 2 == 0), stop=(t == 15),
                            skip_group_check=True,
                        )
                    if e % 2 == 1 or e == ET - 1:
                        j = e // 2
                        w = min(2 * H, (ET - 2 * j) * H)
                        msb = wp.tile([P, 2 * H], BF16, tag="msb", bufs=5, name=f"msb1_{j}")
                        nc.scalar.activation(
                            out=msb[:, 0:w], in_=msg_ps[j][:, 0:w], func=AF.Copy
                        )
                        msbs.append(msb)

                agg_ps = [
                    psA.tile([P, 2 * H], F32, space="PSUM", tag=f"agg{j}", name=f"agg1_{j}")
                    for j in range(NT // 2)
                ]

                def a1(n):
                    return agg_ps[n // 2][:, (n % 2) * H : (n % 2) * H + H]


                ones_sb = cp.tile([1, P], BF16)
                nc.vector.memset(ones_sb[:], 1.0)

                def scatter_root(aget, msbs_l, root_lhs, bias_rhs):
                    for bi, (e, n) in enumerate(sc_blocks):
                        nc.tensor.matmul(
                            aget(n), lhsT=scm_sb[:, P * bi : P * (bi + 1)],
                            rhs=msbs_l[e // 2][:, (e % 2) * H : (e % 2) * H + H],
                            start=(first_touch[n // 2] == ("sc", bi)), stop=False,
                            skip_group_check=True,
                        )
                    for n in range(NT):
                        pairs = root_lhs(n)
                        for li, (lhs, rhs) in enumerate(pairs):
                            last = bias_rhs is None and li == len(pairs) - 1
                            nc.tensor.matmul(
                                aget(n), lhsT=lhs, rhs=rhs,
                                start=(first_touch[n // 2] == ("root", n) and li == 0),
                                stop=last, skip_group_check=True,
                            )
                        if bias_rhs is not None:
                            nc.tensor.matmul(
                                aget(n), lhsT=ones_sb[:], rhs=bias_rhs,
                                start=False, stop=True, skip_group_check=True,
                            )

                def root1(n):
                    return [(xshT_sb[:, P * n : P * (n + 1)], r1wb_sb[:])]

                # bias1 is folded into r1wb (row 64 = ones in xshT)
                scatter_root(a1, msbs, root1, None)

                h1sb = cp.tile([P, NT, H], BF16)
                for j in range(NT // 2):
                    nc.scalar.activation(
                        out=h1sb[:, 2 * j : 2 * j + 2, :], in_=agg_ps[j][:, 0 : 2 * H],
                        func=AF.Relu,
                    )

                if upto == "h1":
                    dh = dbg_out("d_h1", [P, NT * H])
                    tmp = wp.tile([P, NT, H], F32, tag="dbgf")
                    nc.vector.tensor_copy(out=tmp[:], in_=h1sb[:])
                    nc.sync.dma_start(
                        out=dh[:].rearrange("p (t o) -> p t o", o=H), in_=tmp[:]
                    )

                # ======== exchange: sendbuf rows via one-hot matmuls -> AllToAll
                snd_ps = [
                    psA.tile([P, 2 * H], F32, space="PSUM", tag=f"msg{j}", name=f"snd_{j}")
                    for j in range((SBT + 1) // 2)
                ]

                def sb_ps(r):
                    return snd_ps[r // 2][:, (r % 2) * H : (r % 2) * H + H]

                sendbuf = cp.tile([P, 2 * ((SBT + 1) // 2), H], BF16)
                for r in range(SBT):
                    for n in range(NT):
                        blk = r * NT + n
                        nc.tensor.matmul(
                            sb_ps(r), lhsT=sel_sb[:, P * blk : P * (blk + 1)],
                            rhs=h1sb[:, n, :], start=(n == 0 and r % 2 == 0),
                            stop=(n == NT - 1), skip_group_check=True,
                        )
                    if r % 2 == 1 or r == SBT - 1:
                        j = r // 2
                        if (SBT - 2 * j) >= 2:
                            nc.scalar.activation(
                                out=sendbuf[:, 2 * j : 2 * j + 2, :],
                                in_=snd_ps[j][:, 0 : 2 * H], func=AF.Copy,
                            )
                        else:
                            nc.scalar.activation(
                                out=sendbuf[:, 2 * j, :], in_=snd_ps[j][:, 0:H],
                                func=AF.Copy,
                            )
                nc.gpsimd.dma_start(
                    out=a2a_in[:].rearrange("(b p) e -> p b e", p=P),
                    in_=sendbuf[:, 0:SBT, :],
                )
                a2a_out = dr.tile([S, H], BF16)
                nc.gpsimd.collective_compute(
                    "AllToAll", ALU.bypass, replica_groups=rg,
                    ins=[a2a_in[:].opt()], outs=[a2a_out[:].opt()],
                )
                h1srcT = cp.tile([P, 2, e_pad], BF16)
                nc.gpsimd.dma_gather(
                    out_ap=h1srcT[:], in_ap=a2a_out[:], idxs_ap=h1src_sb[:],
                    num_idxs=e_pad, num_idxs_reg=e_pad, elem_size=H,
                    transpose=True, single_packet=False,
                )
                # h1shT via PE transposes of h1sb (PE is idle during the
                # AllToAll; alternating psum tags pipeline transpose+copy)
                h1shT = cp.tile([P, 2, NSH], BF16)
                for n in range(NT):
                    for oh in range(2):
                        tsh = psA.tile(
                            [P, P], BF16, space="PSUM", tag=f"agg{(n * 2 + oh) % 2}",
                            name=f"tsh_{n}_{oh}",
                        )
                        nc.tensor.transpose(
                            out=tsh[:], in_=h1sb[:, n, P * oh : P * (oh + 1)],
                            identity=ident_sb[:],
                        )
                        nc.scalar.activation(
                            out=h1shT[:, oh, P * n : P * (n + 1)], in_=tsh[:],
                            func=AF.Copy,
                        )
                # rotated copies for s=1..3: h1rot_r[p,c] = feat[(128c+p+32r)%256]
                # (32-partition aligned chunks -- walrus rejects unaligned
                # partition-offset spans)
                h1rots = [h1srcT]
                for r in range(1, 4):
                    h1r = cp.tile([P, 2, e_pad], BF16, name=f"h1rot{r}")
                    for c in range(2):
                        for d in range(4):
                            t = 32 * (d + r)
                            q, slot = t % P, (c if t < P else 1 - c)
                            nc.scalar.activation(
                                out=h1r[32 * d : 32 * d + 32, c, :],
                                in_=h1srcT[q : q + 32, slot, :], func=AF.Copy,
                            )
                    h1rots.append(h1r)

                if upto == "h1srcT":
                    d1 = dbg_out("d_h1srcT", [P, 2 * e_pad])
                    tmp = wp.tile([P, 2, e_pad], F32, tag="dbgf")
                    nc.vector.tensor_copy(out=tmp[:], in_=h1srcT[:])
                    nc.sync.dma_start(
                        out=d1[:].rearrange("p (c e) -> p c e", c=2), in_=tmp[:]
                    )

                # ======== conv2: 64 blocks, s-major (s=0 first)
                msg2_ps = [
                    psA.tile([P, 2 * H], F32, space="PSUM", tag=f"msg{j}", name=f"msg2_{j}")
                    for j in range((ET + 1) // 2)
                ]

                def m2(e):
                    return msg2_ps[e // 2][:, (e % 2) * H : (e % 2) * H + H]

                if not zb2:
                    for e in range(ET):
                        for ih in range(2):
                            nc.tensor.matmul(
                                m2(e), lhsT=h1srcT[:, ih, P * e : P * (e + 1)],
                                rhs=b2p_sb[:, ih, :], start=(ih == 0 and e % 2 == 0),
                                stop=False, skip_group_check=True,
                            )
                for b in range(64):
                    s2, q2, ih = b // 16, (b % 16) // 2, b % 2
                    srct = h1rots[s2]
                    zt = wp.tile([P, e_pad], BF16, tag="zt", bufs=4)
                    nc.vector.tensor_tensor(
                        out=zt[:], in0=srct[:, ih, :], in1=bcq_sb[:, q2, :], op=ALU.mult
                    )
                    for e in range(ET):
                        nc.tensor.matmul(
                            m2(e), lhsT=zt[:, P * e : P * (e + 1)], rhs=w2p_sb[:, b, :],
                            start=(zb2 and b == 0 and e % 2 == 0), stop=(b == 63),
                            skip_group_check=True,
                        )

                agg2_ps = [
                    psA.tile([P, 2 * H], F32, space="PSUM", tag=f"agg{j}", name=f"agg2_{j}")
                    for j in range(NT // 2)
                ]

                def a2(n):
                    return agg2_ps[n // 2][:, (n % 2) * H : (n % 2) * H + H]

                msbs2 = []
                for j in range((ET + 1) // 2):
                    w = min(2 * H, (ET - 2 * j) * H)
                    msb = wp.tile([P, 2 * H], BF16, tag="msb", bufs=5)
                    nc.scalar.activation(out=msb[:, 0:w], in_=msg2_ps[j][:, 0:w], func=AF.Copy)
                    msbs2.append(msb)

                def root2(n):
                    return [
                        (h1shT[:, kh, P * n : P * (n + 1)], r2wb_sb[:, kh, :])
                        for kh in range(2)
                    ]

                scatter_root(a2, msbs2, root2, None if zb2 else b2sbb_sb[:])

                h2sb = cp.tile([P, NT, H], BF16)
                for j in range(NT // 2):
                    nc.scalar.activation(
                        out=h2sb[:, 2 * j : 2 * j + 2, :], in_=agg2_ps[j][:, 0 : 2 * H],
                        func=AF.Copy,
                    )

                if upto == "h2":
                    dh = dbg_out("d_h2", [P, NT * H])
                    tmp = wp.tile([P, NT, H], F32, tag="dbgf")
                    nc.vector.tensor_copy(out=tmp[:], in_=h2sb[:])
                    nc.sync.dma_start(
                        out=dh[:].rearrange("p (t o) -> p t o", o=H), in_=tmp[:]
                    )

                # ======== pool (transposed, recip folded into scp) + z1T partials
                # meanT_ps[:, oh, g*128:...] = sum_n h2sb[:,n,128oh:].T @ scp_blk(n,g)
                meanT_ps = psA.tile([P, 2, H], F32, space="PSUM", tag="agg0", name="meanT")
                for n in range(NT):
                    for oh in range(2):
                        for g in range(GT):
                            blk = n * GT + g
                            nc.tensor.matmul(
                                meanT_ps[:, oh, P * g : P * (g + 1)],
                                lhsT=h2sb[:, n, P * oh : P * (oh + 1)],
                                rhs=scp_sb[:, P * blk : P * (blk + 1)],
                                start=(n == 0 and oh == 0 and g == 0),
                                stop=(n == NT - 1 and oh == 1 and g == GT - 1),
                                skip_group_check=True,
                            )
                meanT_sb = cp.tile([P, 2, H], BF16)
                nc.scalar.activation(out=meanT_sb[:], in_=meanT_ps[:], func=AF.Copy)
                # z1T[g, m] = sum_h meanT[h, g] * l1w[h, m]  (+ l1b/8 via ones row)
                z1T_ps = psA.tile([P, GT, H // 2], F32, space="PSUM", tag="agg1", name="z1T")
                for g in range(GT):
                    for oh in range(2):
                        nc.tensor.matmul(
                            z1T_ps[:, g, :],
                            lhsT=meanT_sb[:, oh, P * g : P * (g + 1)],
                            rhs=l1wb_sb[:, oh, :],
                            start=(g == 0 and oh == 0),
                            stop=(zl1 and g == GT - 1 and oh == 1),
                            skip_group_check=True,
                        )
                    if not zl1:
                        nc.tensor.matmul(
                            z1T_ps[:, g, :], lhsT=ones_sb[:], rhs=l1brow_sb[:],
                            start=False, stop=(g == GT - 1), skip_group_check=True,
                        )
                z1T = cp.tile([P, GT, H // 2], F32)
                nc.vector.tensor_copy(out=z1T[:], in_=z1T_ps[:])
                rs_in = dr.tile([N_GRAPHS, H // 2], F32)
                nc.sync.dma_start(
                    out=rs_in[:].rearrange("(g p) m -> p g m", p=P), in_=z1T[:]
                )

            # ======== tail: ReduceScatter, local readout, AllGather
            with tc.tile_pool(name="psB", bufs=1, space="PSUM") as psB:
                rs_out = dr.tile([GSH, H // 2], F32)
                nc.gpsimd.collective_compute(
                    "ReduceScatter", ALU.add, replica_groups=rg,
                    ins=[rs_in[:].opt()], outs=[rs_out[:].opt()],
                )
                # ======== local readout of GSH graphs
                rs_sb = cp.tile([GSH, H // 2], F32)
                nc.sync.dma_start(out=rs_sb[:], in_=rs_out[:])
                # fused relu(x) * l2w with free-dim reduction in one DVE op
                prod = wp.tile([GSH, H // 2], F32, tag="t2")
                red = wp.tile([GSH, 1], F32, tag="t3")
                nc.vector.scalar_tensor_tensor(
                    out=prod[:], in0=rs_sb[:], scalar=0.0, in1=l2w_sb[:],
                    op0=ALU.max, op1=ALU.mult, accum_out=red[:],
                )
                osb = wp.tile([GSH, 1], F32, tag="t4")
                nc.scalar.activation(
                    out=osb[:], in_=red[:], func=AF.Sigmoid, bias=l2b_sb[:, 0:1]
                )
                ag_in = dr.tile([GSH, 1], F32)
                nc.sync.dma_start(out=ag_in[:], in_=osb[:])
                ag_out = dr.tile([N_GRAPHS, 1], F32, addr_space="Shared")
                nc.gpsimd.collective_compute(
                    "AllGather", ALU.bypass, replica_groups=rg,
                    ins=[ag_in[:].opt()], outs=[ag_out[:].opt()],
                )
                nc.sync.dma_start(out=out[:], in_=ag_out[:])

    nc.compile()
    return nc


def _prep_inputs(inputs):
    x = np.asarray(inputs["x"], dtype=np.float32)
    ei = np.asarray(inputs["edge_index"])
    attr = np.asarray(inputs["edge_attr"], dtype=np.float32)
    batch = np.asarray(inputs["batch"]).astype(np.int64)
    src, dst = ei[0].astype(np.int64), ei[1].astype(np.int64)

    owner = dst // NSH
    per_core = []
    for c in range(NCORES):
        eids = np.nonzero(owner == c)[0]
        eids = eids[np.argsort(dst[eids], kind="stable")]
        per_core.append(eids)
    need = max(max(len(e) for e in per_core), 1)
    e_pad = max(((need + P - 1) // P) * P, P)
    ET = e_pad // P

    # static union of scatter blocks (e_tile, n_tile)
    blocks = set()
    for c in range(NCORES):
        dstl = dst[per_core[c]] - c * NSH
        for e in range(ET):
            seg = dstl[e * P : (e + 1) * P]
            if len(seg) == 0:
                continue
            for n in range(int(seg.min()) // P, int(seg.max()) // P + 1):
                blocks.add((e, int(n)))
    sc_blocks = sorted(blocks)
    NSC = len(sc_blocks)

    # A2A send rows (dedup per (sender c, receiver d) pair) and receive mapping
    send_rows = [[None] * NCORES for _ in range(NCORES)]
    recv_pos_parts = [[None] * NCORES for _ in range(NCORES)]  # [d][c]
    maxrows = 1
    for d in range(NCORES):
        eids = per_core[d]
        srcs = src[eids]
        co = srcs // NSH
        for c in range(NCORES):
            mask = co == c
            uniq, inv = np.unique(srcs[mask] - c * NSH, return_inverse=True)
            send_rows[c][d] = uniq
            recv_pos_parts[d][c] = (np.nonzero(mask)[0], inv)
            maxrows = max(maxrows, len(uniq))
    SB = ((maxrows + 15) // 16) * 16
    S = NCORES * SB

    # host-permuted weights (shared)
    nn1_w = np.asarray(inputs["nn1_w"], np.float32)  # [32, 64*256]
    nn2_w = np.asarray(inputs["nn2_w"], np.float32)  # [32, 256*256]
    pidx = np.arange(P)
    g32 = pidx // 32
    j32 = pidx % 32
    nn1_r = nn1_w.reshape(DE, DN, H)
    w1p = np.zeros((P, 16, H), np.float32)
    for t in range(16):
        q, s = t // 2, t % 2
        k = 4 * q + g32
        i = (32 * (g32 + s) + j32) % DN
        w1p[:, t, :] = nn1_r[k, i, :]
    w1p = w1p.astype(BF)
    nn2_r = nn2_w.reshape(DE, H, H)
    w2p = np.zeros((P, 64, H), np.float32)
    for b in range(64):
        s, q, ih = b // 16, (b % 16) // 2, b % 2
        k = 4 * q + g32
        i = (128 * ih + 32 * (g32 + s) + j32) % H
        w2p[:, b, :] = nn2_r[k, i, :]
    w2p = w2p.astype(BF)

    nn1_b = np.asarray(inputs["nn1_b"], np.float32).reshape(DN, H)
    nn2_b = np.asarray(inputs["nn2_b"], np.float32).reshape(H, H)
    b2p = np.stack([nn2_b[0:P, :], nn2_b[P : 2 * P, :]], axis=1)  # [128, 2, 256]
    r1w = np.asarray(inputs["root1_w"], np.float32)
    bias1 = np.asarray(inputs["bias1"], np.float32)
    r1wb = np.concatenate([r1w, bias1.reshape(1, H)], axis=0)  # [65, 256]
    r2w = np.asarray(inputs["root2_w"], np.float32)
    r2wb = np.stack([r2w[0:P, :], r2w[P : 2 * P, :]], axis=1)  # [128, 2, 256]
    bias2 = np.asarray(inputs["bias2"], np.float32).reshape(1, H)
    l1w = np.asarray(inputs["lin1_w"], np.float32)  # [256, 128]
    l1wb = np.stack([l1w[0:P, :], l1w[P : 2 * P, :]], axis=1)  # [128, 2, 128]
    l1b = np.asarray(inputs["lin1_b"], np.float32).reshape(1, H // 2)
    l2w = np.asarray(inputs["lin2_w"], np.float32).reshape(1, H // 2)
    l2b = np.asarray(inputs["lin2_b"], np.float32).reshape(1, 1)
    GSH = N_GRAPHS // NCORES

    cnt = np.bincount(batch, minlength=N_GRAPHS).astype(np.float32)
    recip_g = 1.0 / np.maximum(cnt, 1.0)  # [256], per graph

    common = {
        "w1p": w1p, "w2p": w2p,
        "b1p": nn1_b.astype(BF), "b2p": b2p.astype(BF),
        "r1wb": r1wb.astype(BF), "r2wb": r2wb.astype(BF),
        "b2sbb": bias2.astype(BF),
        "l1wb": l1wb.astype(BF), "l1brow": (l1b / NCORES).astype(BF),
        "l2wrep": np.tile(l2w, (GSH, 1)).astype(np.float32),
        "l2brep": np.tile(l2b, (GSH, 1)).astype(np.float32),
        "identb": np.eye(P, dtype=BF),
    }

    in_maps = []
    for c in range(NCORES):
        eids = per_core[c]
        ne = len(eids)
        srcs = src[eids]
        dstl = (dst[eids] - c * NSH).astype(np.int64)

        xg = x[srcs, :].astype(BF)  # [ne, 64]
        xsrc2 = np.zeros((P, 2, e_pad), BF)
        for s in range(2):
            iofs = (32 * (g32 + s) + j32) % DN  # [128]
            xsrc2[:, s, 0:ne] = xg[:, iofs].T

        ag = attr[eids, :]  # [ne, 32]
        bcq = np.zeros((P, 8, e_pad), BF)
        for q in range(8):
            for g in range(4):
                bcq[32 * g : 32 * g + 32, q, 0:ne] = ag[:, 4 * q + g].astype(BF)[None, :]

        scm = np.zeros((P, NSC * P), BF)
        for bi, (e, n) in enumerate(sc_blocks):
            seg = dstl[e * P : min((e + 1) * P, ne)]
            for p, dv in enumerate(seg):
                q = dv - n * P
                if 0 <= q < P:
                    scm[p, bi * P + q] = 1.0

        batch_l = batch[c * NSH : (c + 1) * NSH]
        scp = np.zeros((P, NT * GT * P), BF)
        for n in range(NT):
            for g in range(GT):
                blk = n * GT + g
                bseg = batch_l[n * P : (n + 1) * P]
                for p, bv in enumerate(bseg):
                    q = bv - g * P
                    if 0 <= q < P:
                        scp[p, blk * P + q] = BF(recip_g[bv])

        xshT = np.ones((DN + 1, NSH), BF)
        xshT[0:DN, :] = x[c * NSH : (c + 1) * NSH, :].astype(BF).T

        snd_idx = np.full(S, -1, np.int64)
        for d in range(NCORES):
            rows = send_rows[c][d]
            snd_idx[d * SB : d * SB + len(rows)] = rows
        SBT = S // P
        selm = np.zeros((P, SBT * NT * P), BF)
        for row in range(S):
            v = snd_idx[row]
            if v < 0:
                continue
            r, q = row // P, row % P
            nt_, npart = int(v) // P, int(v) % P
            selm[npart, (r * NT + nt_) * P + q] = 1.0
        h1src_idx = np.zeros(e_pad, np.int16)
        for d2 in range(NCORES):
            pos, inv = recv_pos_parts[c][d2]
            h1src_idx[pos] = d2 * SB + inv

        m = dict(common)
        m["xsrc2"] = xsrc2
        m["bcq"] = bcq
        m["scm"] = scm
        m["scp"] = scp
        m["sel"] = selm
        m["xshT"] = xshT
        m["h1src_w"] = _wrap_idx(h1src_idx, e_pad)
        in_maps.append(m)

    zb = (
        bool(np.all(np.asarray(inputs["nn1_b"]) == 0)),
        bool(np.all(np.asarray(inputs["nn2_b"]) == 0))
        and bool(np.all(np.asarray(inputs["bias2"]) == 0)),
        bool(np.all(np.asarray(inputs["lin1_b"]) == 0)),
    )
    _PREP["args"] = (e_pad, S, tuple(sc_blocks), zb)
    return e_pad, in_maps


def kernel(**inputs) -> np.ndarray:
    e_pad, in_maps = _prep_inputs(inputs)
    if e_pad not in _cache:
        ep, S, blocks, zb = _PREP["args"]
        _cache[e_pad] = _build(ep, S, list(blocks), zb=zb)
    nc = _cache[e_pad]
    res = bass_utils.run_bass_kernel_spmd(nc, in_maps, core_ids=list(range(NCORES)))
    return np.asarray(res.results[0]["out"], dtype=np.float32)


def run_debug(upto, **inputs):
    e_pad, in_maps = _prep_inputs(inputs)
    ep, S, blocks, zb = _PREP["args"]
    nc = _build(ep, S, list(blocks), zb=zb, upto=upto)
    res = bass_utils.run_bass_kernel_spmd(nc, in_maps, core_ids=list(range(NCORES)))
    return e_pad, res



# revision 9
# speedup vs baseline: 1.2475x; 1.2475x over previous
"""Trainium2 Bass kernel for nn_NNModel2 (2x NNConv GNN + pooled MLP readout).

Self-contained: accepts FULL inputs, shards across 8 NeuronCores, returns the
FULL [256, 1] output.

v3 design (tail collectives removed):
  - Node ranges are GRAPH-ALIGNED (host picks graph cuts near node multiples
    of N/8), so every graph's nodes live on exactly one core. The pooled
    readout is then fully local; each core writes its own [GW, 1] slice and
    the host concatenates. This removes the ReduceScatter + AllGather tail.
  - conv layers use the z-trick: z[e,(k,i)] = attr[e,k]*x[src,i]; msg = z @ W'
    as PSUM-accumulated matmuls over 128-row (k,i) blocks.
  - h1 exchange: AllToAll of deduped per-(src-owner, dst-owner) rows, then a
    dma_gather (transpose) per edge plus partition-rotated copies for the
    conv2 s=1..3 blocks (rotation copies run on Pool, hidden under conv2).
"""

import sys

sys.path.insert(0, "/opt/trn_rl_repo")

import numpy as np
import ml_dtypes

from concourse import bacc, bass, mybir
import concourse.tile as tile
from concourse import bass_utils

P = 128
NCORES = 8
N_NODES = 4096
N_EDGES = 8192
N_GRAPHS = 256
DN = 64
DE = 32
H = 256

F32 = mybir.dt.float32
BF16 = mybir.dt.bfloat16
I16 = mybir.dt.int16
AF = mybir.ActivationFunctionType
ALU = mybir.AluOpType
BF = ml_dtypes.bfloat16

_cache = {}
_PREP = {}


def _wrap_idx(idx, n):
    idx = np.asarray(idx, dtype=np.int16)
    assert idx.shape == (n,) and n % 16 == 0
    return np.tile(idx.reshape(n // 16, 16).T, (8, 1)).copy()


def _build(e_pad, S, sc_blocks, NT, GW, zb=(False, False, False), upto="full"):
    ET = e_pad // P
    SBT = S // P  # send-buffer tiles
    NSH = NT * P
    nc = bacc.Bacc(num_devices=NCORES)

    # ---- per-core inputs (host-prepped)
    xsrc2 = nc.dram_tensor("xsrc2", [P, 2, e_pad], BF16, kind="ExternalInput")
    bcq = nc.dram_tensor("bcq", [P, 8, e_pad], BF16, kind="ExternalInput")
    scm = nc.dram_tensor("scm", [P, len(sc_blocks) * P], BF16, kind="ExternalInput")
    scp = nc.dram_tensor("scp", [P, NT * GW], BF16, kind="ExternalInput")
    sel = nc.dram_tensor("sel", [P, SBT * NT * P], BF16, kind="ExternalInput")
    xshT = nc.dram_tensor("xshT", [DN + 1, NSH], BF16, kind="ExternalInput")
    h1src_w = nc.dram_tensor("h1src_w", [P, e_pad // 16], I16, kind="ExternalInput")
    identb = nc.dram_tensor("identb", [P, P], BF16, kind="ExternalInput")
    # ---- shared weights (host-permuted, bf16)
    w1p = nc.dram_tensor("w1p", [P, 16, H], BF16, kind="ExternalInput")
    w2p = nc.dram_tensor("w2p", [P, 64, H], BF16, kind="ExternalInput")
    b1p = nc.dram_tensor("b1p", [DN, H], BF16, kind="ExternalInput")
    b2p = nc.dram_tensor("b2p", [P, 2, H], BF16, kind="ExternalInput")
    r1wb = nc.dram_tensor("r1wb", [DN + 1, H], BF16, kind="ExternalInput")
    r2wb = nc.dram_tensor("r2wb", [P, 2, H], BF16, kind="ExternalInput")
    b2sbb = nc.dram_tensor("b2sbb", [1, H], BF16, kind="ExternalInput")
    l1wb = nc.dram_tensor("l1wb", [P, 2, H // 2], BF16, kind="ExternalInput")
    l1bcol = nc.dram_tensor("l1bcol", [H // 2, 1], F32, kind="ExternalInput")
    l2wcol = nc.dram_tensor("l2wcol", [H // 2, 1], F32, kind="ExternalInput")
    l2bcol = nc.dram_tensor("l2bcol", [GW, 1], F32, kind="ExternalInput")
    out = nc.dram_tensor("out", [GW, 1], F32, kind="ExternalOutput")

    zb1, zb2, _ = zb
    rg = [list(range(NCORES))]
    NSC = len(sc_blocks)
    NAT = (NT + 1) // 2  # agg psum tiles

    # first bank-touch bookkeeping for agg scatter (bank = n // 2)
    first_touch = {}
    for bi, (e, n) in enumerate(sc_blocks):
        first_touch.setdefault(n // 2, ("sc", bi))
    for n in range(NT):
        first_touch.setdefault(n // 2, ("root", n))

    with tile.TileContext(nc, num_cores=NCORES) as tc:
        with (
            tc.tile_pool(name="const", bufs=1) as cp,
            tc.tile_pool(name="work", bufs=3) as wp,
            tc.tile_pool(name="dram", bufs=1, space="DRAM") as dr,
        ):
            # ======== stage A: loads (SP queue), conv1-critical first.
            bcq_sb = cp.tile([P, 8, e_pad], BF16)
            nc.sync.dma_start(out=bcq_sb[:, 0:2, :], in_=bcq[:, 0:2, :])
            xsrc2_sb = cp.tile([P, 2, e_pad], BF16)
            nc.sync.dma_start(out=xsrc2_sb[:, 0:1, :], in_=xsrc2[:, 0:1, :])
            w1p_sb = cp.tile([P, 16, H], BF16)
            nc.sync.dma_start(out=w1p_sb[:, 0:4, :], in_=w1p[:, 0:4, :])
            nc.sync.dma_start(out=xsrc2_sb[:, 1:2, :], in_=xsrc2[:, 1:2, :])
            b1p_sb = cp.tile([DN, H], BF16)
            nc.sync.dma_start(out=b1p_sb[:], in_=b1p[:])
            for c in range(1, 4):
                nc.sync.dma_start(
                    out=bcq_sb[:, 2 * c : 2 * c + 2, :], in_=bcq[:, 2 * c : 2 * c + 2, :]
                )
                if c == 1:
                    nc.sync.dma_start(out=w1p_sb[:, 4:8, :], in_=w1p[:, 4:8, :])
                if c == 2:
                    nc.sync.dma_start(out=w1p_sb[:, 8:16, :], in_=w1p[:, 8:16, :])
            scm_sb = cp.tile([P, NSC * P], BF16)
            nc.sync.dma_start(out=scm_sb[:], in_=scm[:])
            xshT_sb = cp.tile([DN + 1, NSH], BF16)
            nc.sync.dma_start(out=xshT_sb[:], in_=xshT[:])
            r1wb_sb = cp.tile([DN + 1, H], BF16)
            nc.sync.dma_start(out=r1wb_sb[:], in_=r1wb[:])
            sel_sb = cp.tile([P, SBT * NT * P], BF16)
            nc.sync.dma_start(out=sel_sb[:], in_=sel[:])
            h1src_sb = cp.tile([P, e_pad // 16], I16)
            nc.sync.dma_start(out=h1src_sb[:], in_=h1src_w[:])
            ident_sb = cp.tile([P, P], BF16)
            nc.sync.dma_start(out=ident_sb[:], in_=identb[:])
            # conv2/tail loads last (small ones first, then the big w2p)
            a2a_in = dr.tile([S, H], BF16)
            b2p_sb = cp.tile([P, 2, H], BF16)
            nc.sync.dma_start(out=b2p_sb[:], in_=b2p[:])
            r2wb_sb = cp.tile([P, 2, H], BF16)
            nc.sync.dma_start(out=r2wb_sb[:], in_=r2wb[:])
            b2sbb_sb = cp.tile([1, H], BF16)
            nc.sync.dma_start(out=b2sbb_sb[:], in_=b2sbb[:])
            scp_sb = cp.tile([P, NT * GW], BF16)
            nc.sync.dma_start(out=scp_sb[:], in_=scp[:])
            l1wb_sb = cp.tile([P, 2, H // 2], BF16)
            nc.sync.dma_start(out=l1wb_sb[:], in_=l1wb[:])
            l1b_sb = cp.tile([H // 2, 1], F32)
            nc.sync.dma_start(out=l1b_sb[:], in_=l1bcol[:])
            l2w_sb = cp.tile([H // 2, 1], F32)
            nc.sync.dma_start(out=l2w_sb[:], in_=l2wcol[:])
            l2b_sb = cp.tile([GW, 1], F32)
            nc.sync.dma_start(out=l2b_sb[:], in_=l2bcol[:])
            w2p_sb = cp.tile([P, 64, H], BF16)
            for c in range(4):
                nc.sync.dma_start(
                    out=w2p_sb[:, 16 * c : 16 * c + 16, :],
                    in_=w2p[:, 16 * c : 16 * c + 16, :],
                )

            with tc.tile_pool(name="psA", bufs=1, space="PSUM") as psA:
                # ======== conv1
                msg_ps = [
                    psA.tile([P, 2 * H], F32, space="PSUM", tag=f"msg{j}", name=f"msg1_{j}")
                    for j in range((ET + 1) // 2)
                ]

                def m1(e):
                    return msg_ps[e // 2][:, (e % 2) * H : (e % 2) * H + H]

                msbs = []

                zts1 = []
                for t in range(16):
                    q1, s1 = t // 2, t % 2
                    zt = wp.tile([P, e_pad], BF16, tag=f"zt1_{t}", name=f"zt1_{t}", bufs=1)
                    nc.vector.tensor_tensor(
                        out=zt[:], in0=xsrc2_sb[:, s1, :], in1=bcq_sb[:, q1, :],
                        op=ALU.mult,
                    )
                    zts1.append(zt)
                for e in range(ET):
                    if not zb1:
                        nc.tensor.matmul(
                            m1(e), lhsT=xsrc2_sb[0:DN, 0, P * e : P * (e + 1)],
                            rhs=b1p_sb[:], start=(e % 2 == 0), stop=False,
                            skip_group_check=True,
                        )
                    for t in range(16):
                        nc.tensor.matmul(
                            m1(e), lhsT=zts1[t][:, P * e : P * (e + 1)],
                            rhs=w1p_sb[:, t, :],
                            start=(zb1 and t == 0 and e % 2 == 0), stop=(t == 15),
                            skip_group_check=True,
                        )
                    if e % 2 == 1 or e == ET - 1:
                        j = e // 2
                        w = min(2 * H, (ET - 2 * j) * H)
                        msb = wp.tile([P, 2 * H], BF16, tag="msb", bufs=5, name=f"msb1_{j}")
                        nc.scalar.activation(
                            out=msb[:, 0:w], in_=msg_ps[j][:, 0:w], func=AF.Copy
                        )
                        msbs.append(msb)

                agg_ps = [
                    psA.tile([P, 2 * H], F32, space="PSUM", tag=f"agg{j}", name=f"agg1_{j}")
                    for j in range(NAT)
                ]

                def a1(n):
                    return agg_ps[n // 2][:, (n % 2) * H : (n % 2) * H + H]

                ones_sb = cp.tile([1, P], BF16)
                nc.vector.memset(ones_sb[:], 1.0)

                def scatter_root(aget, msbs_l, root_lhs, bias_rhs):
                    for bi, (e, n) in enumerate(sc_blocks):
                        nc.tensor.matmul(
                            aget(n), lhsT=scm_sb[:, P * bi : P * (bi + 1)],
                            rhs=msbs_l[e // 2][:, (e % 2) * H : (e % 2) * H + H],
                            start=(first_touch[n // 2] == ("sc", bi)), stop=False,
                            skip_group_check=True,
                        )
                    for n in range(NT):
                        pairs = root_lhs(n)
                        for li, (lhs, rhs) in enumerate(pairs):
                            last = bias_rhs is None and li == len(pairs) - 1
                            nc.tensor.matmul(
                                aget(n), lhsT=lhs, rhs=rhs,
                                start=(first_touch[n // 2] == ("root", n) and li == 0),
                                stop=last, skip_group_check=True,
                            )
                        if bias_rhs is not None:
                            nc.tensor.matmul(
                                aget(n), lhsT=ones_sb[:], rhs=bias_rhs,
                                start=False, stop=True, skip_group_check=True,
                            )

                def root1(n):
                    return [(xshT_sb[:, P * n : P * (n + 1)], r1wb_sb[:])]

                # bias1 is folded into r1wb (row 64 = ones in xshT)
                scatter_root(a1, msbs, root1, None)

                h1sb = cp.tile([P, NT, H], BF16)
                for n in range(NT):
                    nc.scalar.activation(
                        out=h1sb[:, n, :], in_=a1(n), func=AF.Relu,
                    )

                if upto == "h1":
                    dh = nc.dram_tensor("d_h1", [P, NT * H], F32, kind="ExternalOutput")
                    tmp = wp.tile([P, NT, H], F32, tag="dbgf")
                    nc.vector.tensor_copy(out=tmp[:], in_=h1sb[:])
                    nc.sync.dma_start(
                        out=dh[:].rearrange("p (t o) -> p t o", o=H), in_=tmp[:]
                    )

                # ======== exchange: sendbuf rows via one-hot matmuls -> AllToAll
                snd_ps = [
                    psA.tile([P, 2 * H], F32, space="PSUM", tag=f"msg{j}", name=f"snd_{j}")
                    for j in range((SBT + 1) // 2)
                ]

                def sb_ps(r):
                    return snd_ps[r // 2][:, (r % 2) * H : (r % 2) * H + H]

                sendbuf = cp.tile([P, 2 * ((SBT + 1) // 2), H], BF16)
                for r in range(SBT):
                    for n in range(NT):
                        blk = r * NT + n
                        nc.tensor.matmul(
                            sb_ps(r), lhsT=sel_sb[:, P * blk : P * (blk + 1)],
                            rhs=h1sb[:, n, :], start=(n == 0 and r % 2 == 0),
                            stop=(n == NT - 1), skip_group_check=True,
                        )
                    if r % 2 == 1 or r == SBT - 1:
                        j = r // 2
                        if (SBT - 2 * j) >= 2:
                            nc.scalar.activation(
                                out=sendbuf[:, 2 * j : 2 * j + 2, :],
                                in_=snd_ps[j][:, 0 : 2 * H], func=AF.Copy,
                            )
                        else:
                            nc.scalar.activation(
                                out=sendbuf[:, 2 * j, :], in_=snd_ps[j][:, 0:H],
                                func=AF.Copy,
                            )
                nc.gpsimd.dma_start(
                    out=a2a_in[:].rearrange("(b p) e -> p b e", p=P),
                    in_=sendbuf[:, 0:SBT, :],
                )
                a2a_out = dr.tile([S, H], BF16)
                nc.gpsimd.collective_compute(
                    "AllToAll", ALU.bypass, replica_groups=rg,
                    ins=[a2a_in[:].opt()], outs=[a2a_out[:].opt()],
                )
                h1srcT = cp.tile([P, 2, e_pad], BF16)
                nc.gpsimd.dma_gather(
                    out_ap=h1srcT[:], in_ap=a2a_out[:], idxs_ap=h1src_sb[:],
                    num_idxs=e_pad, num_idxs_reg=e_pad, elem_size=H,
                    transpose=True, single_packet=False,
                )
                # h1shT via PE transposes of h1sb (PE is idle during the
                # AllToAll; alternating psum tags pipeline transpose+copy)
                h1shT = cp.tile([P, 2, NSH], BF16)
                for n in range(NT):
                    for oh in range(2):
                        tsh = psA.tile(
                            [P, P], BF16, space="PSUM", tag=f"agg{(n * 2 + oh) % 2}",
                            name=f"tsh_{n}_{oh}",
                        )
                        nc.tensor.transpose(
                            out=tsh[:], in_=h1sb[:, n, P * oh : P * (oh + 1)],
                            identity=ident_sb[:],
                        )
                        nc.scalar.activation(
                            out=h1shT[:, oh, P * n : P * (n + 1)], in_=tsh[:],
                            func=AF.Copy,
                        )
                # rotated copies for s=1..3 on the (otherwise idle) Pool engine
                h1rots = [h1srcT]
                for r in range(1, 4):
                    h1r = cp.tile([P, 2, e_pad], BF16, name=f"h1rot{r}")
                    for c in range(2):
                        for d in range(4):
                            t = 32 * (d + r)
                            q, slot = t % P, (c if t < P else 1 - c)
                            nc.gpsimd.tensor_copy(
                                out=h1r[32 * d : 32 * d + 32, c, :],
                                in_=h1srcT[q : q + 32, slot, :],
                            )
                    h1rots.append(h1r)

                if upto == "h1srcT":
                    d1 = nc.dram_tensor("d_h1srcT", [P, 2 * e_pad], F32, kind="ExternalOutput")
                    tmp = wp.tile([P, 2, e_pad], F32, tag="dbgf")
                    nc.vector.tensor_copy(out=tmp[:], in_=h1srcT[:])
                    nc.sync.dma_start(
                        out=d1[:].rearrange("p (c e) -> p c e", c=2), in_=tmp[:]
                    )

                # ======== conv2: 64 blocks, s-major (s=0 first)
                msg2_ps = [
                    psA.tile([P, 2 * H], F32, space="PSUM", tag=f"msg{j}", name=f"msg2_{j}")
                    for j in range((ET + 1) // 2)
                ]

                def m2(e):
                    return msg2_ps[e // 2][:, (e % 2) * H : (e % 2) * H + H]

                if not zb2:
                    for e in range(ET):
                        for ih in range(2):
                            nc.tensor.matmul(
                                m2(e), lhsT=h1srcT[:, ih, P * e : P * (e + 1)],
                                rhs=b2p_sb[:, ih, :], start=(ih == 0 and e % 2 == 0),
                                stop=False, skip_group_check=True,
                            )
                for b in range(64):
                    s2, q2, ih = b // 16, (b % 16) // 2, b % 2
                    srct = h1rots[s2]
                    zt = wp.tile([P, e_pad], BF16, tag="zt", bufs=4)
                    nc.vector.tensor_tensor(
                        out=zt[:], in0=srct[:, ih, :], in1=bcq_sb[:, q2, :], op=ALU.mult
                    )
                    for e in range(ET):
                        nc.tensor.matmul(
                            m2(e), lhsT=zt[:, P * e : P * (e + 1)], rhs=w2p_sb[:, b, :],
                            start=(zb2 and b == 0 and e % 2 == 0), stop=(b == 63),
                            skip_group_check=True,
                        )

                agg2_ps = [
                    psA.tile([P, 2 * H], F32, space="PSUM", tag=f"agg{j}", name=f"agg2_{j}")
                    for j in range(NAT)
                ]

                def a2(n):
                    return agg2_ps[n // 2][:, (n % 2) * H : (n % 2) * H + H]

                msbs2 = []
                for j in range((ET + 1) // 2):
                    w = min(2 * H, (ET - 2 * j) * H)
                    msb = wp.tile([P, 2 * H], BF16, tag="msb", bufs=5)
                    nc.scalar.activation(out=msb[:, 0:w], in_=msg2_ps[j][:, 0:w], func=AF.Copy)
                    msbs2.append(msb)

                def root2(n):
                    return [
                        (h1shT[:, kh, P * n : P * (n + 1)], r2wb_sb[:, kh, :])
                        for kh in range(2)
                    ]

                scatter_root(a2, msbs2, root2, None if zb2 else b2sbb_sb[:])

                h2sb = cp.tile([P, NT, H], BF16)
                for n in range(NT):
                    nc.scalar.activation(
                        out=h2sb[:, n, :], in_=a2(n), func=AF.Copy,
                    )

                if upto == "h2":
                    dh = nc.dram_tensor("d_h2", [P, NT * H], F32, kind="ExternalOutput")
                    tmp = wp.tile([P, NT, H], F32, tag="dbgf")
                    nc.vector.tensor_copy(out=tmp[:], in_=h2sb[:])
                    nc.sync.dma_start(
                        out=dh[:].rearrange("p (t o) -> p t o", o=H), in_=tmp[:]
                    )

                # ======== pool + readout (fully local; graphs are core-owned)
                # meanT[f, g] = sum_n h2sb[:, n].T @ scp_blk(n)  (recip in scp)
                meanT_ps = psA.tile([P, 2, GW], F32, space="PSUM", tag="agg0", name="meanT")
                for n in range(NT):
                    for oh in range(2):
                        nc.tensor.matmul(
                            meanT_ps[:, oh, :],
                            lhsT=h2sb[:, n, P * oh : P * (oh + 1)],
                            rhs=scp_sb[:, GW * n : GW * (n + 1)],
                            start=(n == 0 and oh == 0), stop=(n == NT - 1),
                            skip_group_check=True,
                        )
                meanT_sb = cp.tile([P, 2, GW], BF16)
                nc.scalar.activation(out=meanT_sb[:], in_=meanT_ps[:], func=AF.Copy)
                if upto == "meanT":
                    dm = nc.dram_tensor("d_meanT", [P, 2 * GW], F32, kind="ExternalOutput")
                    tmp = wp.tile([P, 2, GW], F32, tag="dbgf")
                    nc.vector.tensor_copy(out=tmp[:], in_=meanT_ps[:])
                    nc.sync.dma_start(
                        out=dm[:].rearrange("p (c g) -> p c g", c=2), in_=tmp[:]
                    )
                # z1T[m, g] = sum_f meanT[f, g] * l1w[f, m]
                z1T_ps = psA.tile([P, GW], F32, space="PSUM", tag="agg1", name="z1T")
                for oh in range(2):
                    nc.tensor.matmul(
                        z1T_ps[:],
                        lhsT=l1wb_sb[:, oh, :],
                        rhs=meanT_sb[:, oh, :],
                        start=(oh == 0), stop=(oh == 1),
                        skip_group_check=True,
                    )
                # relu(z1 + l1b) with per-partition bias, f32
                z1r = cp.tile([P, GW], F32)
                nc.scalar.activation(
                    out=z1r[:], in_=z1T_ps[:], func=AF.Relu, bias=l1b_sb[:]
                )
                # out[g] = sigmoid(sum_m z1r[m, g] * l2w[m] + l2b)
                o_ps = psA.tile([GW, 1], F32, space="PSUM", tag="agg2", name="oput")
                nc.tensor.matmul(
                    o_ps[:], lhsT=z1r[:], rhs=l2w_sb[:],
                    start=True, stop=True, skip_group_check=True,
                )
                osb = wp.tile([GW, 1], F32, tag="t4")
                nc.scalar.activation(
                    out=osb[:], in_=o_ps[:], func=AF.Sigmoid, bias=l2b_sb[:]
                )
                nc.sync.dma_start(out=out[:], in_=osb[:])

    nc.compile()
    return nc


def _prep_inputs(inputs):
    x = np.asarray(inputs["x"], dtype=np.float32)
    ei = np.asarray(inputs["edge_index"])
    attr = np.asarray(inputs["edge_attr"], dtype=np.float32)
    batch = np.asarray(inputs["batch"]).astype(np.int64)
    src, dst = ei[0].astype(np.int64), ei[1].astype(np.int64)

    # ---- graph-aligned node ranges (cut at graph starts nearest c*N/8)
    gstart = np.searchsorted(batch, np.arange(N_GRAPHS + 1))  # node start per graph
    cuts = [0]
    for c in range(1, NCORES):
        cuts.append(int(np.argmin(np.abs(gstart - (N_NODES // NCORES) * c))))
    cuts.append(N_GRAPHS)
    nr = [int(gstart[cuts[c]]) for c in range(NCORES + 1)]  # node range starts
    node_cnt = [nr[c + 1] - nr[c] for c in range(NCORES)]
    NT = (max(node_cnt) + P - 1) // P
    NSH = NT * P
    win = [cuts[c + 1] - cuts[c] for c in range(NCORES)]
    GW = ((max(win) + 15) // 16) * 16

    owner_of = np.searchsorted(np.asarray(nr[1:]), dst, side="right")
    per_core = []
    for c in range(NCORES):
        eids = np.nonzero(owner_of == c)[0]
        eids = eids[np.argsort(dst[eids], kind="stable")]
        per_core.append(eids)
    need = max(max(len(e) for e in per_core), 1)
    e_pad = max(((need + P - 1) // P) * P, P)
    ET = e_pad // P

    src_owner = np.searchsorted(np.asarray(nr[1:]), src, side="right")

    # static union of scatter blocks (e_tile, n_tile)
    blocks = set()
    for c in range(NCORES):
        dstl = dst[per_core[c]] - nr[c]
        for e in range(ET):
            seg = dstl[e * P : (e + 1) * P]
            if len(seg) == 0:
                continue
            for n in range(int(seg.min()) // P, int(seg.max()) // P + 1):
                blocks.add((e, int(n)))
    sc_blocks = sorted(blocks)
    NSC = len(sc_blocks)

    # A2A send rows (dedup per (sender c, receiver d) pair) and receive mapping
    send_rows = [[None] * NCORES for _ in range(NCORES)]
    recv_pos_parts = [[None] * NCORES for _ in range(NCORES)]  # [d][c]
    maxrows = 1
    for d in range(NCORES):
        eids = per_core[d]
        srcs = src[eids]
        co = src_owner[eids]
        for c in range(NCORES):
            mask = co == c
            uniq, inv = np.unique(srcs[mask] - nr[c], return_inverse=True)
            send_rows[c][d] = uniq
            recv_pos_parts[d][c] = (np.nonzero(mask)[0], inv)
            maxrows = max(maxrows, len(uniq))
    SB = ((maxrows + 15) // 16) * 16
    S = NCORES * SB

    # host-permuted weights (shared)
    nn1_w = np.asarray(inputs["nn1_w"], np.float32)  # [32, 64*256]
    nn2_w = np.asarray(inputs["nn2_w"], np.float32)  # [32, 256*256]
    pidx = np.arange(P)
    g32 = pidx // 32
    j32 = pidx % 32
    nn1_r = nn1_w.reshape(DE, DN, H)
    w1p = np.zeros((P, 16, H), np.float32)
    for t in range(16):
        q, s = t // 2, t % 2
        k = 4 * q + g32
        i = (32 * (g32 + s) + j32) % DN
        w1p[:, t, :] = nn1_r[k, i, :]
    w1p = w1p.astype(BF)
    nn2_r = nn2_w.reshape(DE, H, H)
    w2p = np.zeros((P, 64, H), np.float32)
    for b in range(64):
        s, q, ih = b // 16, (b % 16) // 2, b % 2
        k = 4 * q + g32
        i = (128 * ih + 32 * (g32 + s) + j32) % H
        w2p[:, b, :] = nn2_r[k, i, :]
    w2p = w2p.astype(BF)

    nn1_b = np.asarray(inputs["nn1_b"], np.float32).reshape(DN, H)
    nn2_b = np.asarray(inputs["nn2_b"], np.float32).reshape(H, H)
    b2p = np.stack([nn2_b[0:P, :], nn2_b[P : 2 * P, :]], axis=1)  # [128, 2, 256]
    r1w = np.asarray(inputs["root1_w"], np.float32)
    bias1 = np.asarray(inputs["bias1"], np.float32)
    r1wb = np.concatenate([r1w, bias1.reshape(1, H)], axis=0)  # [65, 256]
    r2w = np.asarray(inputs["root2_w"], np.float32)
    r2wb = np.stack([r2w[0:P, :], r2w[P : 2 * P, :]], axis=1)  # [128, 2, 256]
    bias2 = np.asarray(inputs["bias2"], np.float32).reshape(1, H)
    l1w = np.asarray(inputs["lin1_w"], np.float32)  # [256, 128]
    l1wb = np.stack([l1w[0:P, :], l1w[P : 2 * P, :]], axis=1)  # [128, 2, 128]
    l1b = np.asarray(inputs["lin1_b"], np.float32).reshape(H // 2, 1)
    l2w = np.asarray(inputs["lin2_w"], np.float32).reshape(H // 2, 1)
    l2b = float(np.asarray(inputs["lin2_b"], np.float32).reshape(()))

    cnt = np.bincount(batch, minlength=N_GRAPHS).astype(np.float32)
    recip_g = 1.0 / np.maximum(cnt, 1.0)  # [256], per graph

    common = {
        "w1p": w1p, "w2p": w2p,
        "b1p": nn1_b.astype(BF), "b2p": b2p.astype(BF),
        "r1wb": r1wb.astype(BF), "r2wb": r2wb.astype(BF),
        "b2sbb": bias2.astype(BF),
        "l1wb": l1wb.astype(BF),
        "l1bcol": l1b.astype(np.float32),
        "l2wcol": l2w.astype(np.float32),
        "l2bcol": np.full((GW, 1), l2b, np.float32),
        "identb": np.eye(P, dtype=BF),
    }

    in_maps = []
    for c in range(NCORES):
        eids = per_core[c]
        ne = len(eids)
        srcs = src[eids]
        dstl = (dst[eids] - nr[c]).astype(np.int64)

        xg = x[srcs, :].astype(BF)  # [ne, 64]
        xsrc2 = np.zeros((P, 2, e_pad), BF)
        for s in range(2):
            iofs = (32 * (g32 + s) + j32) % DN  # [128]
            xsrc2[:, s, 0:ne] = xg[:, iofs].T

        ag = attr[eids, :]  # [ne, 32]
        bcq = np.zeros((P, 8, e_pad), BF)
        for q in range(8):
            for g in range(4):
                bcq[32 * g : 32 * g + 32, q, 0:ne] = ag[:, 4 * q + g].astype(BF)[None, :]

        scm = np.zeros((P, NSC * P), BF)
        for bi, (e, n) in enumerate(sc_blocks):
            seg = dstl[e * P : min((e + 1) * P, ne)]
            for p, dv in enumerate(seg):
                q = dv - n * P
                if 0 <= q < P:
                    scm[p, bi * P + q] = 1.0

        # pool scatter: node (local) -> graph (local window), recip weight
        batch_l = batch[nr[c] : nr[c + 1]] - cuts[c]
        gl = batch[nr[c] : nr[c + 1]]
        scp = np.zeros((P, NT * GW), BF)
        for p_loc in range(nr[c + 1] - nr[c]):
            n_t, p_p = p_loc // P, p_loc % P
            scp[p_p, n_t * GW + batch_l[p_loc]] = BF(recip_g[gl[p_loc]])

        xshT = np.ones((DN + 1, NSH), BF)
        xshT[0:DN, :] = 0.0
        xshT[0:DN, 0 : nr[c + 1] - nr[c]] = x[nr[c] : nr[c + 1], :].astype(BF).T
        xshT[DN, nr[c + 1] - nr[c] :] = 0.0

        snd_idx = np.full(S, -1, np.int64)
        for d in range(NCORES):
            rows = send_rows[c][d]
            snd_idx[d * SB : d * SB + len(rows)] = rows
        SBT = S // P
        selm = np.zeros((P, SBT * NT * P), BF)
        for row in range(S):
            v = snd_idx[row]
            if v < 0:
                continue
            r, q = row // P, row % P
            nt_, npart = int(v) // P, int(v) % P
            selm[npart, (r * NT + nt_) * P + q] = 1.0
        h1src_idx = np.zeros(e_pad, np.int16)
        for d2 in range(NCORES):
            pos, inv = recv_pos_parts[c][d2]
            h1src_idx[pos] = d2 * SB + inv

        m = dict(common)
        m["xsrc2"] = xsrc2
        m["bcq"] = bcq
        m["scm"] = scm
        m["scp"] = scp
        m["sel"] = selm
        m["xshT"] = xshT
        m["h1src_w"] = _wrap_idx(h1src_idx, e_pad)
        in_maps.append(m)

    zb = (
        bool(np.all(np.asarray(inputs["nn1_b"]) == 0)),
        bool(np.all(np.asarray(inputs["nn2_b"]) == 0))
        and bool(np.all(np.asarray(inputs["bias2"]) == 0)),
        bool(np.all(np.asarray(inputs["lin1_b"]) == 0)),
    )
    _PREP["args"] = (e_pad, S, tuple(sc_blocks), NT, GW, zb)
    _PREP["cuts"] = cuts
    return e_pad, in_maps


def run_debug(upto, **inputs):
    e_pad, in_maps = _prep_inputs(inputs)
    ep, S, blocks, NT, GW, zb = _PREP["args"]
    nc = _build(ep, S, list(blocks), NT, GW, zb=zb, upto=upto)
    res = bass_utils.run_bass_kernel_spmd(nc, in_maps, core_ids=list(range(NCORES)))
    return e_pad, res


def kernel(**inputs) -> np.ndarray:
    e_pad, in_maps = _prep_inputs(inputs)
    key = _PREP["args"][:5]
    if key not in _cache:
        ep, S, blocks, NT, GW, zb = _PREP["args"]
        _cache[key] = _build(ep, S, list(blocks), NT, GW, zb=zb)
        _cache[e_pad] = _cache[key]  # test.py compat (keyed by e_pad)
    nc = _cache[key]
    res = bass_utils.run_bass_kernel_spmd(nc, in_maps, core_ids=list(range(NCORES)))
    cuts = _PREP["cuts"]
    out = np.zeros((N_GRAPHS, 1), np.float32)
    for c in range(NCORES):
        w = cuts[c + 1] - cuts[c]
        out[cuts[c] : cuts[c + 1], :] = np.asarray(
            res.results[c]["out"], dtype=np.float32
        )[0:w, :]
    return out


# revision 13
# speedup vs baseline: 1.2499x; 1.0019x over previous
"""Trainium2 Bass kernel for nn_NNModel2 (2x NNConv GNN + pooled MLP readout).

Self-contained: accepts FULL inputs, shards across 8 NeuronCores, returns the
FULL [256, 1] output.

v3 design (tail collectives removed):
  - Node ranges are GRAPH-ALIGNED (host picks graph cuts near node multiples
    of N/8), so every graph's nodes live on exactly one core. The pooled
    readout is then fully local; each core writes its own [GW, 1] slice and
    the host concatenates. This removes the ReduceScatter + AllGather tail.
  - conv layers use the z-trick: z[e,(k,i)] = attr[e,k]*x[src,i]; msg = z @ W'
    as PSUM-accumulated matmuls over 128-row (k,i) blocks.
  - h1 exchange: AllToAll of deduped per-(src-owner, dst-owner) rows, then a
    dma_gather (transpose) per edge plus partition-rotated copies for the
    conv2 s=1..3 blocks (rotation copies run on Pool, hidden under conv2).
"""

import sys

sys.path.insert(0, "/opt/trn_rl_repo")

import numpy as np
import ml_dtypes

from concourse import bacc, bass, mybir
import concourse.tile as tile
from concourse import bass_utils

P = 128
NCORES = 8
N_NODES = 4096
N_EDGES = 8192
N_GRAPHS = 256
DN = 64
DE = 32
H = 256

F32 = mybir.dt.float32
BF16 = mybir.dt.bfloat16
I16 = mybir.dt.int16
AF = mybir.ActivationFunctionType
ALU = mybir.AluOpType
BF = ml_dtypes.bfloat16

_cache = {}
_PREP = {}


def _wrap_idx(idx, n):
    idx = np.asarray(idx, dtype=np.int16)
    assert idx.shape == (n,) and n % 16 == 0
    return np.tile(idx.reshape(n // 16, 16).T, (8, 1)).copy()


def _build(e_pad, S, sc_blocks, NT, GW, zb=(False, False, False), upto="full"):
    ET = e_pad // P
    SBT = S // P  # send-buffer tiles
    NSH = NT * P
    nc = bacc.Bacc(num_devices=NCORES)

    # ---- per-core inputs (host-prepped)
    xsrc2 = nc.dram_tensor("xsrc2", [P, 2, e_pad], BF16, kind="ExternalInput")
    bcq = nc.dram_tensor("bcq", [P, 8, e_pad], BF16, kind="ExternalInput")
    scm = nc.dram_tensor("scm", [P, len(sc_blocks) * P], BF16, kind="ExternalInput")
    scp = nc.dram_tensor("scp", [P, NT * GW], BF16, kind="ExternalInput")
    sel = nc.dram_tensor("sel", [P, SBT * NT * P], BF16, kind="ExternalInput")
    xshT = nc.dram_tensor("xshT", [DN + 1, NSH], BF16, kind="ExternalInput")
    h1src_w = nc.dram_tensor("h1src_w", [P, e_pad // 16], I16, kind="ExternalInput")
    identb = nc.dram_tensor("identb", [P, P], BF16, kind="ExternalInput")
    # ---- shared weights (host-permuted, bf16)
    w1p = nc.dram_tensor("w1p", [P, 16, H], BF16, kind="ExternalInput")
    w2p = nc.dram_tensor("w2p", [P, 64, H], BF16, kind="ExternalInput")
    b1p = nc.dram_tensor("b1p", [DN, H], BF16, kind="ExternalInput")
    b2p = nc.dram_tensor("b2p", [P, 2, H], BF16, kind="ExternalInput")
    r1wb = nc.dram_tensor("r1wb", [DN + 1, H], BF16, kind="ExternalInput")
    r2wb = nc.dram_tensor("r2wb", [P, 2, H], BF16, kind="ExternalInput")
    b2sbb = nc.dram_tensor("b2sbb", [1, H], BF16, kind="ExternalInput")
    l1wb = nc.dram_tensor("l1wb", [P, 2, H // 2], BF16, kind="ExternalInput")
    l1bcol = nc.dram_tensor("l1bcol", [H // 2, 1], F32, kind="ExternalInput")
    l2wcol = nc.dram_tensor("l2wcol", [H // 2, 1], F32, kind="ExternalInput")
    l2bcol = nc.dram_tensor("l2bcol", [GW, 1], F32, kind="ExternalInput")
    out = nc.dram_tensor("out", [GW, 1], F32, kind="ExternalOutput")

    zb1, zb2, _ = zb
    rg = [list(range(NCORES))]
    NSC = len(sc_blocks)
    NAT = (NT + 1) // 2  # agg psum tiles

    # first bank-touch bookkeeping for agg scatter (bank = n // 2)
    first_touch = {}
    for bi, (e, n) in enumerate(sc_blocks):
        first_touch.setdefault(n // 2, ("sc", bi))
    for n in range(NT):
        first_touch.setdefault(n // 2, ("root", n))

    with tile.TileContext(nc, num_cores=NCORES) as tc:
        with (
            tc.tile_pool(name="const", bufs=1) as cp,
            tc.tile_pool(name="work", bufs=3) as wp,
            tc.tile_pool(name="dram", bufs=1, space="DRAM") as dr,
        ):
            # ======== stage A: loads (SP queue), conv1-critical first.
            # Fine-grained interleave so the first z-blocks start ~1.7us in:
            # t=2q+s consumes bcq[:,q] and xsrc2[:,s].
            xsrc2_sb = cp.tile([P, 2, e_pad], BF16)
            nc.sync.dma_start(out=xsrc2_sb[:, 0:1, :], in_=xsrc2[:, 0:1, :])
            bcq_sb = cp.tile([P, 8, e_pad], BF16)
            nc.sync.dma_start(out=bcq_sb[:, 0:1, :], in_=bcq[:, 0:1, :])
            w1p_sb = cp.tile([P, 16, H], BF16)
            nc.sync.dma_start(out=w1p_sb[:, 0:4, :], in_=w1p[:, 0:4, :])
            nc.sync.dma_start(out=xsrc2_sb[:, 1:2, :], in_=xsrc2[:, 1:2, :])
            nc.sync.dma_start(out=bcq_sb[:, 1:2, :], in_=bcq[:, 1:2, :])
            b1p_sb = cp.tile([DN, H], BF16)
            nc.sync.dma_start(out=b1p_sb[:], in_=b1p[:])
            for c in range(1, 4):
                nc.sync.dma_start(
                    out=bcq_sb[:, 2 * c : 2 * c + 2, :], in_=bcq[:, 2 * c : 2 * c + 2, :]
                )
                if c == 1:
                    nc.sync.dma_start(out=w1p_sb[:, 4:8, :], in_=w1p[:, 4:8, :])
                if c == 2:
                    nc.sync.dma_start(out=w1p_sb[:, 8:16, :], in_=w1p[:, 8:16, :])
            scm_sb = cp.tile([P, NSC * P], BF16)
            nc.sync.dma_start(out=scm_sb[:], in_=scm[:])
            xshT_sb = cp.tile([DN + 1, NSH], BF16)
            nc.sync.dma_start(out=xshT_sb[:], in_=xshT[:])
            r1wb_sb = cp.tile([DN + 1, H], BF16)
            nc.sync.dma_start(out=r1wb_sb[:], in_=r1wb[:])
            sel_sb = cp.tile([P, SBT * NT * P], BF16)
            nc.sync.dma_start(out=sel_sb[:], in_=sel[:])
            h1src_sb = cp.tile([P, e_pad // 16], I16)
            nc.sync.dma_start(out=h1src_sb[:], in_=h1src_w[:])
            ident_sb = cp.tile([P, P], BF16)
            nc.sync.dma_start(out=ident_sb[:], in_=identb[:])
            # conv2/tail loads last (small ones first, then the big w2p)
            a2a_in = dr.tile([S, H], BF16)
            b2p_sb = cp.tile([P, 2, H], BF16)
            nc.sync.dma_start(out=b2p_sb[:], in_=b2p[:])
            r2wb_sb = cp.tile([P, 2, H], BF16)
            nc.sync.dma_start(out=r2wb_sb[:], in_=r2wb[:])
            b2sbb_sb = cp.tile([1, H], BF16)
            nc.sync.dma_start(out=b2sbb_sb[:], in_=b2sbb[:])
            scp_sb = cp.tile([P, NT * GW], BF16)
            nc.sync.dma_start(out=scp_sb[:], in_=scp[:])
            l1wb_sb = cp.tile([P, 2, H // 2], BF16)
            nc.sync.dma_start(out=l1wb_sb[:], in_=l1wb[:])
            l1b_sb = cp.tile([H // 2, 1], F32)
            nc.sync.dma_start(out=l1b_sb[:], in_=l1bcol[:])
            l2w_sb = cp.tile([H // 2, 1], F32)
            nc.sync.dma_start(out=l2w_sb[:], in_=l2wcol[:])
            l2b_sb = cp.tile([GW, 1], F32)
            nc.sync.dma_start(out=l2b_sb[:], in_=l2bcol[:])
            w2p_sb = cp.tile([P, 64, H], BF16)
            for c in range(4):
                nc.sync.dma_start(
                    out=w2p_sb[:, 16 * c : 16 * c + 16, :],
                    in_=w2p[:, 16 * c : 16 * c + 16, :],
                )

            with tc.tile_pool(name="psA", bufs=1, space="PSUM") as psA:
                # ======== conv1
                msg_ps = [
                    psA.tile([P, 2 * H], F32, space="PSUM", tag=f"msg{j}", name=f"msg1_{j}")
                    for j in range((ET + 1) // 2)
                ]

                def m1(e):
                    return msg_ps[e // 2][:, (e % 2) * H : (e % 2) * H + H]

                msbs = []

                zts1 = []
                for t in range(16):
                    q1, s1 = t // 2, t % 2
                    zt = wp.tile([P, e_pad], BF16, tag=f"zt1_{t}", name=f"zt1_{t}", bufs=1)
                    nc.vector.tensor_tensor(
                        out=zt[:], in0=xsrc2_sb[:, s1, :], in1=bcq_sb[:, q1, :],
                        op=ALU.mult,
                    )
                    zts1.append(zt)
                # hybrid order: t-major for the first T1 blocks (pipelines with
                # the initial DMA loads), then e-major so psum banks close
                # incrementally for the ACT copies.
                T1 = 4
                if not zb1:
                    for e in range(ET):
                        nc.tensor.matmul(
                            m1(e), lhsT=xsrc2_sb[0:DN, 0, P * e : P * (e + 1)],
                            rhs=b1p_sb[:], start=(e % 2 == 0), stop=False,
                            skip_group_check=True,
                        )
                for t in range(T1):
                    for e in range(ET):
                        nc.tensor.matmul(
                            m1(e), lhsT=zts1[t][:, P * e : P * (e + 1)],
                            rhs=w1p_sb[:, t, :],
                            start=(zb1 and t == 0 and e % 2 == 0), stop=False,
                            skip_group_check=True,
                        )
                for e in range(ET):
                    for t in range(T1, 16):
                        nc.tensor.matmul(
                            m1(e), lhsT=zts1[t][:, P * e : P * (e + 1)],
                            rhs=w1p_sb[:, t, :],
                            start=False, stop=(t == 15),
                            skip_group_check=True,
                        )
                    if e % 2 == 1 or e == ET - 1:
                        j = e // 2
                        w = min(2 * H, (ET - 2 * j) * H)
                        msb = wp.tile([P, 2 * H], BF16, tag="msb", bufs=5, name=f"msb1_{j}")
                        nc.scalar.activation(
                            out=msb[:, 0:w], in_=msg_ps[j][:, 0:w], func=AF.Copy
                        )
                        msbs.append(msb)

                agg_ps = [
                    psA.tile([P, 2 * H], F32, space="PSUM", tag=f"agg{j}", name=f"agg1_{j}")
                    for j in range(NAT)
                ]

                def a1(n):
                    return agg_ps[n // 2][:, (n % 2) * H : (n % 2) * H + H]

                ones_sb = cp.tile([1, P], BF16)
                nc.vector.memset(ones_sb[:], 1.0)

                def scatter_root(aget, msbs_l, root_lhs, bias_rhs):
                    for bi, (e, n) in enumerate(sc_blocks):
                        nc.tensor.matmul(
                            aget(n), lhsT=scm_sb[:, P * bi : P * (bi + 1)],
                            rhs=msbs_l[e // 2][:, (e % 2) * H : (e % 2) * H + H],
                            start=(first_touch[n // 2] == ("sc", bi)), stop=False,
                            skip_group_check=True,
                        )
                    for n in range(NT):
                        pairs = root_lhs(n)
                        for li, (lhs, rhs) in enumerate(pairs):
                            last = bias_rhs is None and li == len(pairs) - 1
                            nc.tensor.matmul(
                                aget(n), lhsT=lhs, rhs=rhs,
                                start=(first_touch[n // 2] == ("root", n) and li == 0),
                                stop=last, skip_group_check=True,
                            )
                        if bias_rhs is not None:
                            nc.tensor.matmul(
                                aget(n), lhsT=ones_sb[:], rhs=bias_rhs,
                                start=False, stop=True, skip_group_check=True,
                            )

                def root1(n):
                    return [(xshT_sb[:, P * n : P * (n + 1)], r1wb_sb[:])]

                # bias1 is folded into r1wb (row 64 = ones in xshT)
                scatter_root(a1, msbs, root1, None)

                h1sb = cp.tile([P, NT, H], BF16)
                for n in range(NT):
                    nc.scalar.activation(
                        out=h1sb[:, n, :], in_=a1(n), func=AF.Relu,
                    )

                if upto == "h1":
                    dh = nc.dram_tensor("d_h1", [P, NT * H], F32, kind="ExternalOutput")
                    tmp = wp.tile([P, NT, H], F32, tag="dbgf")
                    nc.vector.tensor_copy(out=tmp[:], in_=h1sb[:])
                    nc.sync.dma_start(
                        out=dh[:].rearrange("p (t o) -> p t o", o=H), in_=tmp[:]
                    )

                # ======== exchange: sendbuf rows via one-hot matmuls -> AllToAll
                snd_ps = [
                    psA.tile([P, 2 * H], F32, space="PSUM", tag=f"msg{j}", name=f"snd_{j}")
                    for j in range((SBT + 1) // 2)
                ]

                def sb_ps(r):
                    return snd_ps[r // 2][:, (r % 2) * H : (r % 2) * H + H]

                sendbuf = cp.tile([P, 2 * ((SBT + 1) // 2), H], BF16)
                for r in range(SBT):
                    for n in range(NT):
                        blk = r * NT + n
                        nc.tensor.matmul(
                            sb_ps(r), lhsT=sel_sb[:, P * blk : P * (blk + 1)],
                            rhs=h1sb[:, n, :], start=(n == 0 and r % 2 == 0),
                            stop=(n == NT - 1), skip_group_check=True,
                        )
                    if r % 2 == 1 or r == SBT - 1:
                        j = r // 2
                        if (SBT - 2 * j) >= 2:
                            nc.scalar.activation(
                                out=sendbuf[:, 2 * j : 2 * j + 2, :],
                                in_=snd_ps[j][:, 0 : 2 * H], func=AF.Copy,
                            )
                        else:
                            nc.scalar.activation(
                                out=sendbuf[:, 2 * j, :], in_=snd_ps[j][:, 0:H],
                                func=AF.Copy,
                            )
                nc.gpsimd.dma_start(
                    out=a2a_in[:].rearrange("(b p) e -> p b e", p=P),
                    in_=sendbuf[:, 0:SBT, :],
                )
                a2a_out = dr.tile([S, H], BF16)
                nc.gpsimd.collective_compute(
                    "AllToAll", ALU.bypass, replica_groups=rg,
                    ins=[a2a_in[:].opt()], outs=[a2a_out[:].opt()],
                )
                h1srcT = cp.tile([P, 2, e_pad], BF16)
                nc.gpsimd.dma_gather(
                    out_ap=h1srcT[:], in_ap=a2a_out[:], idxs_ap=h1src_sb[:],
                    num_idxs=e_pad, num_idxs_reg=e_pad, elem_size=H,
                    transpose=True, single_packet=False,
                )
                # h1shT via PE transposes of h1sb (PE is idle during the
                # AllToAll; alternating psum tags pipeline transpose+copy)
                h1shT = cp.tile([P, 2, NSH], BF16)
                for n in range(NT):
                    for oh in range(2):
                        tsh = psA.tile(
                            [P, P], BF16, space="PSUM", tag=f"agg{(n * 2 + oh) % 2}",
                            name=f"tsh_{n}_{oh}",
                        )
                        nc.tensor.transpose(
                            out=tsh[:], in_=h1sb[:, n, P * oh : P * (oh + 1)],
                            identity=ident_sb[:],
                        )
                        nc.scalar.activation(
                            out=h1shT[:, oh, P * n : P * (n + 1)], in_=tsh[:],
                            func=AF.Copy,
                        )
                # rotated copies for s=1..3 on the (otherwise idle) Pool engine
                h1rots = [h1srcT]
                for r in range(1, 4):
                    h1r = cp.tile([P, 2, e_pad], BF16, name=f"h1rot{r}")
                    for c in range(2):
                        for d in range(4):
                            t = 32 * (d + r)
                            q, slot = t % P, (c if t < P else 1 - c)
                            nc.gpsimd.tensor_copy(
                                out=h1r[32 * d : 32 * d + 32, c, :],
                                in_=h1srcT[q : q + 32, slot, :],
                            )
                    h1rots.append(h1r)

                # ======== root2 early: runs on PE during the AllToAll window.
                # Root-first bank ordering: root2 of the even tile opens each
                # agg2 bank; the scatter closes it later.
                agg2_ps = [
                    psA.tile([P, 2 * H], F32, space="PSUM", tag=f"agg{j}", name=f"agg2_{j}")
                    for j in range(NAT)
                ]

                def a2(n):
                    return agg2_ps[n // 2][:, (n % 2) * H : (n % 2) * H + H]

                for n in range(NT):
                    for kh in range(2):
                        nc.tensor.matmul(
                            a2(n), lhsT=h1shT[:, kh, P * n : P * (n + 1)],
                            rhs=r2wb_sb[:, kh, :],
                            start=(n % 2 == 0 and kh == 0), stop=False,
                            skip_group_check=True,
                        )
                    if not zb2:
                        nc.tensor.matmul(
                            a2(n), lhsT=ones_sb[:], rhs=b2sbb_sb[:],
                            start=False, stop=False, skip_group_check=True,
                        )

                if upto == "h1srcT":
                    d1 = nc.dram_tensor("d_h1srcT", [P, 2 * e_pad], F32, kind="ExternalOutput")
                    tmp = wp.tile([P, 2, e_pad], F32, tag="dbgf")
                    nc.vector.tensor_copy(out=tmp[:], in_=h1srcT[:])
                    nc.sync.dma_start(
                        out=d1[:].rearrange("p (c e) -> p c e", c=2), in_=tmp[:]
                    )

                # ======== conv2: 64 blocks, s-major (s=0 first)
                msg2_ps = [
                    psA.tile([P, 2 * H], F32, space="PSUM", tag=f"msg{j}", name=f"msg2_{j}")
                    for j in range((ET + 1) // 2)
                ]

                def m2(e):
                    return msg2_ps[e // 2][:, (e % 2) * H : (e % 2) * H + H]

                if not zb2:
                    for e in range(ET):
                        for ih in range(2):
                            nc.tensor.matmul(
                                m2(e), lhsT=h1srcT[:, ih, P * e : P * (e + 1)],
                                rhs=b2p_sb[:, ih, :], start=(ih == 0 and e % 2 == 0),
                                stop=False, skip_group_check=True,
                            )
                for b in range(64):
                    s2, q2, ih = b // 16, (b % 16) // 2, b % 2
                    srct = h1rots[s2]
                    zt = wp.tile([P, e_pad], BF16, tag="zt", bufs=4)
                    nc.vector.tensor_tensor(
                        out=zt[:], in0=srct[:, ih, :], in1=bcq_sb[:, q2, :], op=ALU.mult
                    )
                    for e in range(ET):
                        nc.tensor.matmul(
                            m2(e), lhsT=zt[:, P * e : P * (e + 1)], rhs=w2p_sb[:, b, :],
                            start=(zb2 and b == 0 and e % 2 == 0), stop=(b == 63),
                            skip_group_check=True,
                        )

                msbs2 = []
                for j in range((ET + 1) // 2):
                    w = min(2 * H, (ET - 2 * j) * H)
                    msb = wp.tile([P, 2 * H], BF16, tag="msb", bufs=5)
                    nc.scalar.activation(out=msb[:, 0:w], in_=msg2_ps[j][:, 0:w], func=AF.Copy)
                    msbs2.append(msb)

                # scatter only (roots already accumulated); last block per
                # bank closes the accumulation group.
                last_of_bank = {}
                for bi, (e, n) in enumerate(sc_blocks):
                    last_of_bank[n // 2] = bi
                for bi, (e, n) in enumerate(sc_blocks):
                    nc.tensor.matmul(
                        a2(n), lhsT=scm_sb[:, P * bi : P * (bi + 1)],
                        rhs=msbs2[e // 2][:, (e % 2) * H : (e % 2) * H + H],
                        start=False, stop=(last_of_bank[n // 2] == bi),
                        skip_group_check=True,
                    )

                h2sb = cp.tile([P, NT, H], BF16)
                for n in range(NT):
                    nc.scalar.activation(
                        out=h2sb[:, n, :], in_=a2(n), func=AF.Copy,
                    )

                if upto == "h2":
                    dh = nc.dram_tensor("d_h2", [P, NT * H], F32, kind="ExternalOutput")
                    tmp = wp.tile([P, NT, H], F32, tag="dbgf")
                    nc.vector.tensor_copy(out=tmp[:], in_=h2sb[:])
                    nc.sync.dma_start(
                        out=dh[:].rearrange("p (t o) -> p t o", o=H), in_=tmp[:]
                    )

                # ======== pool + readout (fully local; graphs are core-owned)
                # meanT[f, g] = sum_n h2sb[:, n].T @ scp_blk(n)  (recip in scp)
                meanT_ps = psA.tile([P, 2, GW], F32, space="PSUM", tag="agg0", name="meanT")
                for n in range(NT):
                    for oh in range(2):
                        nc.tensor.matmul(
                            meanT_ps[:, oh, :],
                            lhsT=h2sb[:, n, P * oh : P * (oh + 1)],
                            rhs=scp_sb[:, GW * n : GW * (n + 1)],
                            start=(n == 0 and oh == 0), stop=(n == NT - 1),
                            skip_group_check=True,
                        )
                meanT_sb = cp.tile([P, 2, GW], BF16)
                nc.scalar.activation(out=meanT_sb[:], in_=meanT_ps[:], func=AF.Copy)
                if upto == "meanT":
                    dm = nc.dram_tensor("d_meanT", [P, 2 * GW], F32, kind="ExternalOutput")
                    tmp = wp.tile([P, 2, GW], F32, tag="dbgf")
                    nc.vector.tensor_copy(out=tmp[:], in_=meanT_ps[:])
                    nc.sync.dma_start(
                        out=dm[:].rearrange("p (c g) -> p c g", c=2), in_=tmp[:]
                    )
                # z1T[m, g] = sum_f meanT[f, g] * l1w[f, m]
                z1T_ps = psA.tile([P, GW], F32, space="PSUM", tag="agg1", name="z1T")
                for oh in range(2):
                    nc.tensor.matmul(
                        z1T_ps[:],
                        lhsT=l1wb_sb[:, oh, :],
                        rhs=meanT_sb[:, oh, :],
                        start=(oh == 0), stop=(oh == 1),
                        skip_group_check=True,
                    )
                # relu(z1 + l1b) with per-partition bias, f32
                z1r = cp.tile([P, GW], F32)
                nc.scalar.activation(
                    out=z1r[:], in_=z1T_ps[:], func=AF.Relu, bias=l1b_sb[:]
                )
                # out[g] = sigmoid(sum_m z1r[m, g] * l2w[m] + l2b)
                o_ps = psA.tile([GW, 1], F32, space="PSUM", tag="agg2", name="oput")
                nc.tensor.matmul(
                    o_ps[:], lhsT=z1r[:], rhs=l2w_sb[:],
                    start=True, stop=True, skip_group_check=True,
                )
                osb = wp.tile([GW, 1], F32, tag="t4")
                nc.scalar.activation(
                    out=osb[:], in_=o_ps[:], func=AF.Sigmoid, bias=l2b_sb[:]
                )
                nc.sync.dma_start(out=out[:], in_=osb[:])

    nc.compile()
    return nc


def _prep_inputs(inputs):
    x = np.asarray(inputs["x"], dtype=np.float32)
    ei = np.asarray(inputs["edge_index"])
    attr = np.asarray(inputs["edge_attr"], dtype=np.float32)
    batch = np.asarray(inputs["batch"]).astype(np.int64)
    src, dst = ei[0].astype(np.int64), ei[1].astype(np.int64)

    # ---- graph-aligned node ranges (cut at graph starts nearest c*N/8)
    gstart = np.searchsorted(batch, np.arange(N_GRAPHS + 1))  # node start per graph
    cuts = [0]
    for c in range(1, NCORES):
        cuts.append(int(np.argmin(np.abs(gstart - (N_NODES // NCORES) * c))))
    cuts.append(N_GRAPHS)
    nr = [int(gstart[cuts[c]]) for c in range(NCORES + 1)]  # node range starts
    node_cnt = [nr[c + 1] - nr[c] for c in range(NCORES)]
    NT = (max(node_cnt) + P - 1) // P
    NSH = NT * P
    win = [cuts[c + 1] - cuts[c] for c in range(NCORES)]
    GW = ((max(win) + 15) // 16) * 16

    owner_of = np.searchsorted(np.asarray(nr[1:]), dst, side="right")
    per_core = []
    for c in range(NCORES):
        eids = np.nonzero(owner_of == c)[0]
        eids = eids[np.argsort(dst[eids], kind="stable")]
        per_core.append(eids)
    need = max(max(len(e) for e in per_core), 1)
    e_pad = max(((need + P - 1) // P) * P, P)
    ET = e_pad // P

    src_owner = np.searchsorted(np.asarray(nr[1:]), src, side="right")

    # static union of scatter blocks (e_tile, n_tile)
    blocks = set()
    for c in range(NCORES):
        dstl = dst[per_core[c]] - nr[c]
        for e in range(ET):
            seg = dstl[e * P : (e + 1) * P]
            if len(seg) == 0:
                continue
            for n in range(int(seg.min()) // P, int(seg.max()) // P + 1):
                blocks.add((e, int(n)))
    sc_blocks = sorted(blocks)
    NSC = len(sc_blocks)

    # A2A send rows (dedup per (sender c, receiver d) pair) and receive mapping
    send_rows = [[None] * NCORES for _ in range(NCORES)]
    recv_pos_parts = [[None] * NCORES for _ in range(NCORES)]  # [d][c]
    maxrows = 1
    for d in range(NCORES):
        eids = per_core[d]
        srcs = src[eids]
        co = src_owner[eids]
        for c in range(NCORES):
            mask = co == c
            uniq, inv = np.unique(srcs[mask] - nr[c], return_inverse=True)
            send_rows[c][d] = uniq
            recv_pos_parts[d][c] = (np.nonzero(mask)[0], inv)
            maxrows = max(maxrows, len(uniq))
    SB = ((maxrows + 15) // 16) * 16
    S = NCORES * SB

    # host-permuted weights (shared)
    nn1_w = np.asarray(inputs["nn1_w"], np.float32)  # [32, 64*256]
    nn2_w = np.asarray(inputs["nn2_w"], np.float32)  # [32, 256*256]
    pidx = np.arange(P)
    g32 = pidx // 32
    j32 = pidx % 32
    nn1_r = nn1_w.reshape(DE, DN, H)
    w1p = np.zeros((P, 16, H), np.float32)
    for t in range(16):
        q, s = t // 2, t % 2
        k = 4 * q + g32
        i = (32 * (g32 + s) + j32) % DN
        w1p[:, t, :] = nn1_r[k, i, :]
    w1p = w1p.astype(BF)
    nn2_r = nn2_w.reshape(DE, H, H)
    w2p = np.zeros((P, 64, H), np.float32)
    for b in range(64):
        s, q, ih = b // 16, (b % 16) // 2, b % 2
        k = 4 * q + g32
        i = (128 * ih + 32 * (g32 + s) + j32) % H
        w2p[:, b, :] = nn2_r[k, i, :]
    w2p = w2p.astype(BF)

    nn1_b = np.asarray(inputs["nn1_b"], np.float32).reshape(DN, H)
    nn2_b = np.asarray(inputs["nn2_b"], np.float32).reshape(H, H)
    b2p = np.stack([nn2_b[0:P, :], nn2_b[P : 2 * P, :]], axis=1)  # [128, 2, 256]
    r1w = np.asarray(inputs["root1_w"], np.float32)
    bias1 = np.asarray(inputs["bias1"], np.float32)
    r1wb = np.concatenate([r1w, bias1.reshape(1, H)], axis=0)  # [65, 256]
    r2w = np.asarray(inputs["root2_w"], np.float32)
    r2wb = np.stack([r2w[0:P, :], r2w[P : 2 * P, :]], axis=1)  # [128, 2, 256]
    bias2 = np.asarray(inputs["bias2"], np.float32).reshape(1, H)
    l1w = np.asarray(inputs["lin1_w"], np.float32)  # [256, 128]
    l1wb = np.stack([l1w[0:P, :], l1w[P : 2 * P, :]], axis=1)  # [128, 2, 128]
    l1b = np.asarray(inputs["lin1_b"], np.float32).reshape(H // 2, 1)
    l2w = np.asarray(inputs["lin2_w"], np.float32).reshape(H // 2, 1)
    l2b = float(np.asarray(inputs["lin2_b"], np.float32).reshape(()))

    cnt = np.bincount(batch, minlength=N_GRAPHS).astype(np.float32)
    recip_g = 1.0 / np.maximum(cnt, 1.0)  # [256], per graph

    common = {
        "w1p": w1p, "w2p": w2p,
        "b1p": nn1_b.astype(BF), "b2p": b2p.astype(BF),
        "r1wb": r1wb.astype(BF), "r2wb": r2wb.astype(BF),
        "b2sbb": bias2.astype(BF),
        "l1wb": l1wb.astype(BF),
        "l1bcol": l1b.astype(np.float32),
        "l2wcol": l2w.astype(np.float32),
        "l2bcol": np.full((GW, 1), l2b, np.float32),
        "identb": np.eye(P, dtype=BF),
    }

    in_maps = []
    for c in range(NCORES):
        eids = per_core[c]
        ne = len(eids)
        srcs = src[eids]
        dstl = (dst[eids] - nr[c]).astype(np.int64)

        xg = x[srcs, :].astype(BF)  # [ne, 64]
        xsrc2 = np.zeros((P, 2, e_pad), BF)
        for s in range(2):
            iofs = (32 * (g32 + s) + j32) % DN  # [128]
            xsrc2[:, s, 0:ne] = xg[:, iofs].T

        ag = attr[eids, :]  # [ne, 32]
        bcq = np.zeros((P, 8, e_pad), BF)
        for q in range(8):
            for g in range(4):
                bcq[32 * g : 32 * g + 32, q, 0:ne] = ag[:, 4 * q + g].astype(BF)[None, :]

        scm = np.zeros((P, NSC * P), BF)
        for bi, (e, n) in enumerate(sc_blocks):
            seg = dstl[e * P : min((e + 1) * P, ne)]
            for p, dv in enumerate(seg):
                q = dv - n * P
                if 0 <= q < P:
                    scm[p, bi * P + q] = 1.0

        # pool scatter: node (local) -> graph (local window), recip weight
        batch_l = batch[nr[c] : nr[c + 1]] - cuts[c]
        gl = batch[nr[c] : nr[c + 1]]
        scp = np.zeros((P, NT * GW), BF)
        for p_loc in range(nr[c + 1] - nr[c]):
            n_t, p_p = p_loc // P, p_loc % P
            scp[p_p, n_t * GW + batch_l[p_loc]] = BF(recip_g[gl[p_loc]])

        xshT = np.ones((DN + 1, NSH), BF)
        xshT[0:DN, :] = 0.0
        xshT[0:DN, 0 : nr[c + 1] - nr[c]] = x[nr[c] : nr[c + 1], :].astype(BF).T
        xshT[DN, nr[c + 1] - nr[c] :] = 0.0

        snd_idx = np.full(S, -1, np.int64)
        for d in range(NCORES):
            rows = send_rows[c][d]
            snd_idx[d * SB : d * SB + len(rows)] = rows
        SBT = S // P
        selm = np.zeros((P, SBT * NT * P), BF)
        for row in range(S):
            v = snd_idx[row]
            if v < 0:
                continue
            r, q = row // P, row % P
            nt_, npart = int(v) // P, int(v) % P
            selm[npart, (r * NT + nt_) * P + q] = 1.0
        h1src_idx = np.zeros(e_pad, np.int16)
        for d2 in range(NCORES):
            pos, inv = recv_pos_parts[c][d2]
            h1src_idx[pos] = d2 * SB + inv

        m = dict(common)
        m["xsrc2"] = xsrc2
        m["bcq"] = bcq
        m["scm"] = scm
        m["scp"] = scp
        m["sel"] = selm
        m["xshT"] = xshT
        m["h1src_w"] = _wrap_idx(h1src_idx, e_pad)
        in_maps.append(m)

    zb = (
        bool(np.all(np.asarray(inputs["nn1_b"]) == 0)),
        bool(np.all(np.asarray(inputs["nn2_b"]) == 0))
        and bool(np.all(np.asarray(inputs["bias2"]) == 0)),
        bool(np.all(np.asarray(inputs["lin1_b"]) == 0)),
    )
    _PREP["args"] = (e_pad, S, tuple(sc_blocks), NT, GW, zb)
    _PREP["cuts"] = cuts
    return e_pad, in_maps


def run_debug(upto, **inputs):
    e_pad, in_maps = _prep_inputs(inputs)
    ep, S, blocks, NT, GW, zb = _PREP["args"]
    nc = _build(ep, S, list(blocks), NT, GW, zb=zb, upto=upto)
    res = bass_utils.run_bass_kernel_spmd(nc, in_maps, core_ids=list(range(NCORES)))
    return e_pad, res


def kernel(**inputs) -> np.ndarray:
    e_pad, in_maps = _prep_inputs(inputs)
    key = _PREP["args"][:5]
    if key not in _cache:
        ep, S, blocks, NT, GW, zb = _PREP["args"]
        _cache[key] = _build(ep, S, list(blocks), NT, GW, zb=zb)
        _cache[e_pad] = _cache[key]  # test.py compat (keyed by e_pad)
    nc = _cache[key]
    res = bass_utils.run_bass_kernel_spmd(nc, in_maps, core_ids=list(range(NCORES)))
    cuts = _PREP["cuts"]
    out = np.zeros((N_GRAPHS, 1), np.float32)
    for c in range(NCORES):
        w = cuts[c + 1] - cuts[c]
        out[cuts[c] : cuts[c + 1], :] = np.asarray(
            res.results[c]["out"], dtype=np.float32
        )[0:w, :]
    return out


# revision 21
# speedup vs baseline: 1.2513x; 1.0011x over previous
"""Trainium2 Bass kernel for nn_NNModel2 (2x NNConv GNN + pooled MLP readout).

Self-contained: accepts FULL inputs, shards across 8 NeuronCores, returns the
FULL [256, 1] output.

v3 design (tail collectives removed):
  - Node ranges are GRAPH-ALIGNED (host picks graph cuts near node multiples
    of N/8), so every graph's nodes live on exactly one core. The pooled
    readout is then fully local; each core writes its own [GW, 1] slice and
    the host concatenates. This removes the ReduceScatter + AllGather tail.
  - conv layers use the z-trick: z[e,(k,i)] = attr[e,k]*x[src,i]; msg = z @ W'
    as PSUM-accumulated matmuls over 128-row (k,i) blocks.
  - h1 exchange: AllToAll of deduped per-(src-owner, dst-owner) rows, then a
    dma_gather (transpose) per edge plus partition-rotated copies for the
    conv2 s=1..3 blocks (rotation copies run on Pool, hidden under conv2).
"""

import sys

sys.path.insert(0, "/opt/trn_rl_repo")

import numpy as np
import ml_dtypes

from concourse import bacc, bass, mybir
import concourse.tile as tile
from concourse import bass_utils

P = 128
NCORES = 8
N_NODES = 4096
N_EDGES = 8192
N_GRAPHS = 256
DN = 64
DE = 32
H = 256

F32 = mybir.dt.float32
BF16 = mybir.dt.bfloat16
I16 = mybir.dt.int16
AF = mybir.ActivationFunctionType
ALU = mybir.AluOpType
BF = ml_dtypes.bfloat16

_cache = {}
_PREP = {}


def _wrap_idx(idx, n):
    idx = np.asarray(idx, dtype=np.int16)
    assert idx.shape == (n,) and n % 16 == 0
    return np.tile(idx.reshape(n // 16, 16).T, (8, 1)).copy()


def _build(e_pad, S, sc_blocks, NT, GW, zb=(False, False, False), upto="full",
           sel_blocks=None):
    ET = e_pad // P
    SBT = S // P  # send-buffer tiles
    NSH = NT * P
    nc = bacc.Bacc(num_devices=NCORES)

    # ---- per-core inputs (host-prepped)
    xsrc2 = nc.dram_tensor("xsrc2", [P, 2, e_pad], BF16, kind="ExternalInput")
    bcq = nc.dram_tensor("bcq", [P, 8, e_pad], BF16, kind="ExternalInput")
    scm = nc.dram_tensor("scm", [P, len(sc_blocks) * P], BF16, kind="ExternalInput")
    scp = nc.dram_tensor("scp", [P, NT * GW], BF16, kind="ExternalInput")
    sel = nc.dram_tensor("sel", [P, SBT * NT * P], BF16, kind="ExternalInput")
    xshT = nc.dram_tensor("xshT", [DN + 1, NSH], BF16, kind="ExternalInput")
    h1src_w = nc.dram_tensor("h1src_w", [P, e_pad // 16], I16, kind="ExternalInput")
    identb = nc.dram_tensor("identb", [P, P], BF16, kind="ExternalInput")
    # ---- shared weights (host-permuted, bf16)
    w1p = nc.dram_tensor("w1p", [P, 16, H], BF16, kind="ExternalInput")
    w2p = nc.dram_tensor("w2p", [P, 64, H], BF16, kind="ExternalInput")
    b1p = nc.dram_tensor("b1p", [DN, H], BF16, kind="ExternalInput")
    b2p = nc.dram_tensor("b2p", [P, 2, H], BF16, kind="ExternalInput")
    r1wb = nc.dram_tensor("r1wb", [DN + 1, H], BF16, kind="ExternalInput")
    r2wb = nc.dram_tensor("r2wb", [P, 2, H], BF16, kind="ExternalInput")
    b2sbb = nc.dram_tensor("b2sbb", [1, H], BF16, kind="ExternalInput")
    l1wb = nc.dram_tensor("l1wb", [P, 2, H // 2], BF16, kind="ExternalInput")
    l1bcol = nc.dram_tensor("l1bcol", [H // 2, 1], F32, kind="ExternalInput")
    l2wcol = nc.dram_tensor("l2wcol", [H // 2, 1], F32, kind="ExternalInput")
    l2bcol = nc.dram_tensor("l2bcol", [GW, 1], F32, kind="ExternalInput")
    out = nc.dram_tensor("out", [GW, 1], F32, kind="ExternalOutput")

    zb1, zb2, _ = zb
    rg = [list(range(NCORES))]
    NSC = len(sc_blocks)
    NAT = (NT + 1) // 2  # agg psum tiles

    # first bank-touch bookkeeping for agg scatter (bank = n // 2)
    first_touch = {}
    for bi, (e, n) in enumerate(sc_blocks):
        first_touch.setdefault(n // 2, ("sc", bi))
    for n in range(NT):
        first_touch.setdefault(n // 2, ("root", n))

    with tile.TileContext(nc, num_cores=NCORES) as tc:
        with (
            tc.tile_pool(name="const", bufs=1) as cp,
            tc.tile_pool(name="work", bufs=3) as wp,
            tc.tile_pool(name="dram", bufs=1, space="DRAM") as dr,
        ):
            # ======== stage A: loads (SP queue), conv1-critical first.
            # Fine-grained interleave so the first z-blocks start ~1.7us in:
            # t=2q+s consumes bcq[:,q] and xsrc2[:,s].
            xsrc2_sb = cp.tile([P, 2, e_pad], BF16)
            nc.sync.dma_start(out=xsrc2_sb[:, 0:1, :], in_=xsrc2[:, 0:1, :])
            bcq_sb = cp.tile([P, 8, e_pad], BF16)
            nc.sync.dma_start(out=bcq_sb[:, 0:1, :], in_=bcq[:, 0:1, :])
            w1p_sb = cp.tile([P, 16, H], BF16)
            nc.sync.dma_start(out=w1p_sb[:, 0:4, :], in_=w1p[:, 0:4, :])
            nc.sync.dma_start(out=xsrc2_sb[:, 1:2, :], in_=xsrc2[:, 1:2, :])
            nc.sync.dma_start(out=bcq_sb[:, 1:2, :], in_=bcq[:, 1:2, :])
            b1p_sb = cp.tile([DN, H], BF16)
            nc.sync.dma_start(out=b1p_sb[:], in_=b1p[:])
            for c in range(1, 4):
                nc.sync.dma_start(
                    out=bcq_sb[:, 2 * c : 2 * c + 2, :], in_=bcq[:, 2 * c : 2 * c + 2, :]
                )
                if c == 1:
                    nc.sync.dma_start(out=w1p_sb[:, 4:8, :], in_=w1p[:, 4:8, :])
                if c == 2:
                    nc.sync.dma_start(out=w1p_sb[:, 8:16, :], in_=w1p[:, 8:16, :])
            scm_sb = cp.tile([P, NSC * P], BF16)
            nc.sync.dma_start(out=scm_sb[:], in_=scm[:])
            xshT_sb = cp.tile([DN + 1, NSH], BF16)
            nc.sync.dma_start(out=xshT_sb[:], in_=xshT[:])
            r1wb_sb = cp.tile([DN + 1, H], BF16)
            nc.sync.dma_start(out=r1wb_sb[:], in_=r1wb[:])
            sel_sb = cp.tile([P, SBT * NT * P], BF16)
            nc.sync.dma_start(out=sel_sb[:], in_=sel[:])
            h1src_sb = cp.tile([P, e_pad // 16], I16)
            nc.sync.dma_start(out=h1src_sb[:], in_=h1src_w[:])
            ident_sb = cp.tile([P, P], BF16)
            nc.sync.dma_start(out=ident_sb[:], in_=identb[:])
            # conv2/tail loads last (small ones first, then the big w2p)
            a2a_in = dr.tile([S, H], BF16)
            b2p_sb = cp.tile([P, 2, H], BF16)
            nc.sync.dma_start(out=b2p_sb[:], in_=b2p[:])
            r2wb_sb = cp.tile([P, 2, H], BF16)
            nc.sync.dma_start(out=r2wb_sb[:], in_=r2wb[:])
            b2sbb_sb = cp.tile([1, H], BF16)
            nc.sync.dma_start(out=b2sbb_sb[:], in_=b2sbb[:])
            scp_sb = cp.tile([P, NT * GW], BF16)
            nc.sync.dma_start(out=scp_sb[:], in_=scp[:])
            l1wb_sb = cp.tile([P, 2, H // 2], BF16)
            nc.sync.dma_start(out=l1wb_sb[:], in_=l1wb[:])
            l1b_sb = cp.tile([H // 2, 1], F32)
            nc.sync.dma_start(out=l1b_sb[:], in_=l1bcol[:])
            l2w_sb = cp.tile([H // 2, 1], F32)
            nc.sync.dma_start(out=l2w_sb[:], in_=l2wcol[:])
            l2b_sb = cp.tile([GW, 1], F32)
            nc.sync.dma_start(out=l2b_sb[:], in_=l2bcol[:])
            w2p_sb = cp.tile([P, 64, H], BF16)
            for c in range(4):
                nc.sync.dma_start(
                    out=w2p_sb[:, 16 * c : 16 * c + 16, :],
                    in_=w2p[:, 16 * c : 16 * c + 16, :],
                )

            with tc.tile_pool(name="psA", bufs=1, space="PSUM") as psA:
                # ======== conv1
                msg_ps = [
                    psA.tile([P, 2 * H], F32, space="PSUM", tag=f"msg{j}", name=f"msg1_{j}")
                    for j in range((ET + 1) // 2)
                ]

                def m1(e):
                    return msg_ps[e // 2][:, (e % 2) * H : (e % 2) * H + H]

                msbs = []

                zts1 = []
                for t in range(16):
                    q1, s1 = t // 2, t % 2
                    zt = wp.tile([P, e_pad], BF16, tag=f"zt1_{t}", name=f"zt1_{t}", bufs=1)
                    nc.vector.tensor_tensor(
                        out=zt[:], in0=xsrc2_sb[:, s1, :], in1=bcq_sb[:, q1, :],
                        op=ALU.mult,
                    )
                    zts1.append(zt)
                # hybrid order: t-major for the first T1 blocks (pipelines with
                # the initial DMA loads), then e-major so psum banks close
                # incrementally for the ACT copies.
                T1 = 4
                if not zb1:
                    for e in range(ET):
                        nc.tensor.matmul(
                            m1(e), lhsT=xsrc2_sb[0:DN, 0, P * e : P * (e + 1)],
                            rhs=b1p_sb[:], start=(e % 2 == 0), stop=False,
                            skip_group_check=True,
                        )
                for t in range(T1):
                    for e in range(ET):
                        nc.tensor.matmul(
                            m1(e), lhsT=zts1[t][:, P * e : P * (e + 1)],
                            rhs=w1p_sb[:, t, :],
                            start=(zb1 and t == 0 and e % 2 == 0), stop=False,
                            skip_group_check=True,
                        )
                for e in range(ET):
                    for t in range(T1, 16):
                        nc.tensor.matmul(
                            m1(e), lhsT=zts1[t][:, P * e : P * (e + 1)],
                            rhs=w1p_sb[:, t, :],
                            start=False, stop=(t == 15),
                            skip_group_check=True,
                        )
                    if e % 2 == 1 or e == ET - 1:
                        j = e // 2
                        w = min(2 * H, (ET - 2 * j) * H)
                        msb = wp.tile([P, 2 * H], BF16, tag="msb", bufs=5, name=f"msb1_{j}")
                        nc.scalar.activation(
                            out=msb[:, 0:w], in_=msg_ps[j][:, 0:w], func=AF.Copy
                        )
                        msbs.append(msb)

                agg_ps = [
                    psA.tile([P, 2 * H], F32, space="PSUM", tag=f"agg{j}", name=f"agg1_{j}")
                    for j in range(NAT)
                ]

                def a1(n):
                    return agg_ps[n // 2][:, (n % 2) * H : (n % 2) * H + H]

                ones_sb = cp.tile([1, P], BF16)
                nc.vector.memset(ones_sb[:], 1.0)

                def scatter_root(aget, msbs_l, root_lhs, bias_rhs):
                    for bi, (e, n) in enumerate(sc_blocks):
                        nc.tensor.matmul(
                            aget(n), lhsT=scm_sb[:, P * bi : P * (bi + 1)],
                            rhs=msbs_l[e // 2][:, (e % 2) * H : (e % 2) * H + H],
                            start=(first_touch[n // 2] == ("sc", bi)), stop=False,
                            skip_group_check=True,
                        )
                    for n in range(NT):
                        pairs = root_lhs(n)
                        for li, (lhs, rhs) in enumerate(pairs):
                            last = bias_rhs is None and li == len(pairs) - 1
                            nc.tensor.matmul(
                                aget(n), lhsT=lhs, rhs=rhs,
                                start=(first_touch[n // 2] == ("root", n) and li == 0),
                                stop=last, skip_group_check=True,
                            )
                        if bias_rhs is not None:
                            nc.tensor.matmul(
                                aget(n), lhsT=ones_sb[:], rhs=bias_rhs,
                                start=False, stop=True, skip_group_check=True,
                            )

                def root1(n):
                    return [(xshT_sb[:, P * n : P * (n + 1)], r1wb_sb[:])]

                # bias1 is folded into r1wb (row 64 = ones in xshT)
                scatter_root(a1, msbs, root1, None)

                h1sb = cp.tile([P, NT, H], BF16)
                for n in range(NT):
                    nc.scalar.activation(
                        out=h1sb[:, n, :], in_=a1(n), func=AF.Relu,
                    )

                if upto == "h1":
                    dh = nc.dram_tensor("d_h1", [P, NT * H], F32, kind="ExternalOutput")
                    tmp = wp.tile([P, NT, H], F32, tag="dbgf")
                    nc.vector.tensor_copy(out=tmp[:], in_=h1sb[:])
                    nc.sync.dma_start(
                        out=dh[:].rearrange("p (t o) -> p t o", o=H), in_=tmp[:]
                    )

                # ======== exchange: sendbuf rows via one-hot matmuls -> AllToAll
                snd_ps = [
                    psA.tile([P, 2 * H], F32, space="PSUM", tag=f"msg{j}", name=f"snd_{j}")
                    for j in range((SBT + 1) // 2)
                ]

                def sb_ps(r):
                    return snd_ps[r // 2][:, (r % 2) * H : (r % 2) * H + H]

                sendbuf = cp.tile([P, 2 * ((SBT + 1) // 2), H], BF16)
                if sel_blocks is None:
                    sel_nz = {(r, n) for r in range(SBT) for n in range(NT)}
                else:
                    sel_nz = set(sel_blocks)
                for r in range(SBT):
                    rn = sorted(n for (rr, n) in sel_nz if rr == r) or [0]
                    for n in rn:
                        blk = r * NT + n
                        nc.tensor.matmul(
                            sb_ps(r), lhsT=sel_sb[:, P * blk : P * (blk + 1)],
                            rhs=h1sb[:, n, :], start=(n == rn[0] and r % 2 == 0),
                            stop=(n == rn[-1]), skip_group_check=True,
                        )
                    if r % 2 == 1 or r == SBT - 1:
                        j = r // 2
                        if (SBT - 2 * j) >= 2:
                            nc.scalar.activation(
                                out=sendbuf[:, 2 * j : 2 * j + 2, :],
                                in_=snd_ps[j][:, 0 : 2 * H], func=AF.Copy,
                            )
                            nc.gpsimd.dma_start(
                                out=a2a_in[:].rearrange("(b p) e -> p b e", p=P)[
                                    :, 2 * j : 2 * j + 2, :
                                ],
                                in_=sendbuf[:, 2 * j : 2 * j + 2, :],
                            )
                        else:
                            nc.scalar.activation(
                                out=sendbuf[:, 2 * j, :], in_=snd_ps[j][:, 0:H],
                                func=AF.Copy,
                            )
                            nc.gpsimd.dma_start(
                                out=a2a_in[:].rearrange("(b p) e -> p b e", p=P)[
                                    :, 2 * j, :
                                ],
                                in_=sendbuf[:, 2 * j, :],
                            )
                a2a_out = dr.tile([S, H], BF16)
                nc.gpsimd.collective_compute(
                    "AllToAll", ALU.bypass, replica_groups=rg,
                    ins=[a2a_in[:].opt()], outs=[a2a_out[:].opt()],
                )
                h1srcT = cp.tile([P, 2, e_pad], BF16)
                nc.gpsimd.dma_gather(
                    out_ap=h1srcT[:], in_ap=a2a_out[:], idxs_ap=h1src_sb[:],
                    num_idxs=e_pad, num_idxs_reg=e_pad, elem_size=H,
                    transpose=True, single_packet=False,
                )
                # h1shT via PE transposes of h1sb (PE is idle during the
                # AllToAll; alternating psum tags pipeline transpose+copy)
                h1shT = cp.tile([P, 2, NSH], BF16)
                for n in range(NT):
                    for oh in range(2):
                        tsh = psA.tile(
                            [P, P], BF16, space="PSUM", tag=f"agg{(n * 2 + oh) % 2}",
                            name=f"tsh_{n}_{oh}",
                        )
                        nc.tensor.transpose(
                            out=tsh[:], in_=h1sb[:, n, P * oh : P * (oh + 1)],
                            identity=ident_sb[:],
                        )
                        nc.scalar.activation(
                            out=h1shT[:, oh, P * n : P * (n + 1)], in_=tsh[:],
                            func=AF.Copy,
                        )
                # rotated copies for s=1..3 on the (otherwise idle) Pool engine
                h1rots = [h1srcT]
                for r in range(1, 4):
                    h1r = cp.tile([P, 2, e_pad], BF16, name=f"h1rot{r}")
                    for c in range(2):
                        for d in range(4):
                            t = 32 * (d + r)
                            q, slot = t % P, (c if t < P else 1 - c)
                            nc.gpsimd.tensor_copy(
                                out=h1r[32 * d : 32 * d + 32, c, :],
                                in_=h1srcT[q : q + 32, slot, :],
                            )
                    h1rots.append(h1r)

                # ======== root2 early: runs on PE during the AllToAll window.
                # Root-first bank ordering: root2 of the even tile opens each
                # agg2 bank; the scatter closes it later.
                agg2_ps = [
                    psA.tile([P, 2 * H], F32, space="PSUM", tag=f"agg{j}", name=f"agg2_{j}")
                    for j in range(NAT)
                ]

                def a2(n):
                    return agg2_ps[n // 2][:, (n % 2) * H : (n % 2) * H + H]

                for n in range(NT):
                    for kh in range(2):
                        nc.tensor.matmul(
                            a2(n), lhsT=h1shT[:, kh, P * n : P * (n + 1)],
                            rhs=r2wb_sb[:, kh, :],
                            start=(n % 2 == 0 and kh == 0), stop=False,
                            skip_group_check=True,
                        )
                    if not zb2:
                        nc.tensor.matmul(
                            a2(n), lhsT=ones_sb[:], rhs=b2sbb_sb[:],
                            start=False, stop=False, skip_group_check=True,
                        )

                if upto == "h1srcT":
                    d1 = nc.dram_tensor("d_h1srcT", [P, 2 * e_pad], F32, kind="ExternalOutput")
                    tmp = wp.tile([P, 2, e_pad], F32, tag="dbgf")
                    nc.vector.tensor_copy(out=tmp[:], in_=h1srcT[:])
                    nc.sync.dma_start(
                        out=d1[:].rearrange("p (c e) -> p c e", c=2), in_=tmp[:]
                    )

                # ======== conv2: 64 blocks, s-major (s=0 first)
                msg2_ps = [
                    psA.tile([P, 2 * H], F32, space="PSUM", tag=f"msg{j}", name=f"msg2_{j}")
                    for j in range((ET + 1) // 2)
                ]

                def m2(e):
                    return msg2_ps[e // 2][:, (e % 2) * H : (e % 2) * H + H]

                if not zb2:
                    for e in range(ET):
                        for ih in range(2):
                            nc.tensor.matmul(
                                m2(e), lhsT=h1srcT[:, ih, P * e : P * (e + 1)],
                                rhs=b2p_sb[:, ih, :], start=(ih == 0 and e % 2 == 0),
                                stop=False, skip_group_check=True,
                            )
                for b in range(64):
                    s2, q2, ih = b // 16, (b % 16) // 2, b % 2
                    srct = h1rots[s2]
                    zt = wp.tile([P, e_pad], BF16, tag="zt", bufs=4)
                    nc.vector.tensor_tensor(
                        out=zt[:], in0=srct[:, ih, :], in1=bcq_sb[:, q2, :], op=ALU.mult
                    )
                    for e in range(ET):
                        nc.tensor.matmul(
                            m2(e), lhsT=zt[:, P * e : P * (e + 1)], rhs=w2p_sb[:, b, :],
                            start=(zb2 and b == 0 and e % 2 == 0), stop=(b == 63),
                            skip_group_check=True,
                        )

                msbs2 = []
                for j in range((ET + 1) // 2):
                    w = min(2 * H, (ET - 2 * j) * H)
                    msb = wp.tile([P, 2 * H], BF16, tag="msb", bufs=5)
                    nc.scalar.activation(out=msb[:, 0:w], in_=msg2_ps[j][:, 0:w], func=AF.Copy)
                    msbs2.append(msb)

                # scatter only (roots already accumulated); last block per
                # bank closes the accumulation group.
                last_of_bank = {}
                for bi, (e, n) in enumerate(sc_blocks):
                    last_of_bank[n // 2] = bi
                for bi, (e, n) in enumerate(sc_blocks):
                    nc.tensor.matmul(
                        a2(n), lhsT=scm_sb[:, P * bi : P * (bi + 1)],
                        rhs=msbs2[e // 2][:, (e % 2) * H : (e % 2) * H + H],
                        start=False, stop=(last_of_bank[n // 2] == bi),
                        skip_group_check=True,
                    )

                h2sb = cp.tile([P, NT, H], BF16)
                for n in range(NT):
                    nc.scalar.activation(
                        out=h2sb[:, n, :], in_=a2(n), func=AF.Copy,
                    )

                if upto == "h2":
                    dh = nc.dram_tensor("d_h2", [P, NT * H], F32, kind="ExternalOutput")
                    tmp = wp.tile([P, NT, H], F32, tag="dbgf")
                    nc.vector.tensor_copy(out=tmp[:], in_=h2sb[:])
                    nc.sync.dma_start(
                        out=dh[:].rearrange("p (t o) -> p t o", o=H), in_=tmp[:]
                    )

                # ======== pool + readout (fully local; graphs are core-owned)
                # meanT[f, g] = sum_n h2sb[:, n].T @ scp_blk(n)  (recip in scp)
                meanT_ps = psA.tile([P, 2, GW], F32, space="PSUM", tag="agg0", name="meanT")
                for n in range(NT):
                    for oh in range(2):
                        nc.tensor.matmul(
                            meanT_ps[:, oh, :],
                            lhsT=h2sb[:, n, P * oh : P * (oh + 1)],
                            rhs=scp_sb[:, GW * n : GW * (n + 1)],
                            start=(n == 0 and oh == 0), stop=(n == NT - 1),
                            skip_group_check=True,
                        )
                meanT_sb = cp.tile([P, 2, GW], BF16)
                nc.scalar.activation(out=meanT_sb[:], in_=meanT_ps[:], func=AF.Copy)
                if upto == "meanT":
                    dm = nc.dram_tensor("d_meanT", [P, 2 * GW], F32, kind="ExternalOutput")
                    tmp = wp.tile([P, 2, GW], F32, tag="dbgf")
                    nc.vector.tensor_copy(out=tmp[:], in_=meanT_ps[:])
                    nc.sync.dma_start(
                        out=dm[:].rearrange("p (c g) -> p c g", c=2), in_=tmp[:]
                    )
                # z1T[m, g] = sum_f meanT[f, g] * l1w[f, m]
                z1T_ps = psA.tile([P, GW], F32, space="PSUM", tag="agg1", name="z1T")
                for oh in range(2):
                    nc.tensor.matmul(
                        z1T_ps[:],
                        lhsT=l1wb_sb[:, oh, :],
                        rhs=meanT_sb[:, oh, :],
                        start=(oh == 0), stop=(oh == 1),
                        skip_group_check=True,
                    )
                # relu(z1 + l1b) with per-partition bias, f32
                z1r = cp.tile([P, GW], F32)
                nc.scalar.activation(
                    out=z1r[:], in_=z1T_ps[:], func=AF.Relu, bias=l1b_sb[:]
                )
                # out[g] = sigmoid(sum_m z1r[m, g] * l2w[m] + l2b)
                o_ps = psA.tile([GW, 1], F32, space="PSUM", tag="agg2", name="oput")
                nc.tensor.matmul(
                    o_ps[:], lhsT=z1r[:], rhs=l2w_sb[:],
                    start=True, stop=True, skip_group_check=True,
                )
                osb = wp.tile([GW, 1], F32, tag="t4")
                nc.scalar.activation(
                    out=osb[:], in_=o_ps[:], func=AF.Sigmoid, bias=l2b_sb[:]
                )
                nc.sync.dma_start(out=out[:], in_=osb[:])

    nc.compile()
    return nc


def _prep_inputs(inputs):
    x = np.asarray(inputs["x"], dtype=np.float32)
    ei = np.asarray(inputs["edge_index"])
    attr = np.asarray(inputs["edge_attr"], dtype=np.float32)
    batch = np.asarray(inputs["batch"]).astype(np.int64)
    src, dst = ei[0].astype(np.int64), ei[1].astype(np.int64)

    # ---- graph-aligned node ranges (cut at graph starts nearest c*N/8)
    gstart = np.searchsorted(batch, np.arange(N_GRAPHS + 1))  # node start per graph
    cuts = [0]
    for c in range(1, NCORES):
        cuts.append(int(np.argmin(np.abs(gstart - (N_NODES // NCORES) * c))))
    cuts.append(N_GRAPHS)
    nr = [int(gstart[cuts[c]]) for c in range(NCORES + 1)]  # node range starts
    node_cnt = [nr[c + 1] - nr[c] for c in range(NCORES)]
    NT = (max(node_cnt) + P - 1) // P
    NSH = NT * P
    win = [cuts[c + 1] - cuts[c] for c in range(NCORES)]
    GW = ((max(win) + 15) // 16) * 16

    owner_of = np.searchsorted(np.asarray(nr[1:]), dst, side="right")
    per_core = []
    for c in range(NCORES):
        eids = np.nonzero(owner_of == c)[0]
        eids = eids[np.argsort(dst[eids], kind="stable")]
        per_core.append(eids)
    need = max(max(len(e) for e in per_core), 1)
    e_pad = max(((need + P - 1) // P) * P, P)
    ET = e_pad // P

    src_owner = np.searchsorted(np.asarray(nr[1:]), src, side="right")

    # static union of scatter blocks (e_tile, n_tile)
    blocks = set()
    for c in range(NCORES):
        dstl = dst[per_core[c]] - nr[c]
        for e in range(ET):
            seg = dstl[e * P : (e + 1) * P]
            if len(seg) == 0:
                continue
            for n in range(int(seg.min()) // P, int(seg.max()) // P + 1):
                blocks.add((e, int(n)))
    sc_blocks = sorted(blocks)
    NSC = len(sc_blocks)

    # A2A send rows (dedup per (sender c, receiver d) pair) and receive mapping
    send_rows = [[None] * NCORES for _ in range(NCORES)]
    recv_pos_parts = [[None] * NCORES for _ in range(NCORES)]  # [d][c]
    maxrows = 1
    for d in range(NCORES):
        eids = per_core[d]
        srcs = src[eids]
        co = src_owner[eids]
        for c in range(NCORES):
            mask = co == c
            uniq, inv = np.unique(srcs[mask] - nr[c], return_inverse=True)
            send_rows[c][d] = uniq
            recv_pos_parts[d][c] = (np.nonzero(mask)[0], inv)
            maxrows = max(maxrows, len(uniq))
    SB = ((maxrows + 15) // 16) * 16
    S = NCORES * SB

    # host-permuted weights (shared)
    nn1_w = np.asarray(inputs["nn1_w"], np.float32)  # [32, 64*256]
    nn2_w = np.asarray(inputs["nn2_w"], np.float32)  # [32, 256*256]
    pidx = np.arange(P)
    g32 = pidx // 32
    j32 = pidx % 32
    nn1_r = nn1_w.reshape(DE, DN, H)
    w1p = np.zeros((P, 16, H), np.float32)
    for t in range(16):
        q, s = t // 2, t % 2
        k = 4 * q + g32
        i = (32 * (g32 + s) + j32) % DN
        w1p[:, t, :] = nn1_r[k, i, :]
    w1p = w1p.astype(BF)
    nn2_r = nn2_w.reshape(DE, H, H)
    w2p = np.zeros((P, 64, H), np.float32)
    for b in range(64):
        s, q, ih = b // 16, (b % 16) // 2, b % 2
        k = 4 * q + g32
        i = (128 * ih + 32 * (g32 + s) + j32) % H
        w2p[:, b, :] = nn2_r[k, i, :]
    w2p = w2p.astype(BF)

    nn1_b = np.asarray(inputs["nn1_b"], np.float32).reshape(DN, H)
    nn2_b = np.asarray(inputs["nn2_b"], np.float32).reshape(H, H)
    b2p = np.stack([nn2_b[0:P, :], nn2_b[P : 2 * P, :]], axis=1)  # [128, 2, 256]
    r1w = np.asarray(inputs["root1_w"], np.float32)
    bias1 = np.asarray(inputs["bias1"], np.float32)
    r1wb = np.concatenate([r1w, bias1.reshape(1, H)], axis=0)  # [65, 256]
    r2w = np.asarray(inputs["root2_w"], np.float32)
    r2wb = np.stack([r2w[0:P, :], r2w[P : 2 * P, :]], axis=1)  # [128, 2, 256]
    bias2 = np.asarray(inputs["bias2"], np.float32).reshape(1, H)
    l1w = np.asarray(inputs["lin1_w"], np.float32)  # [256, 128]
    l1wb = np.stack([l1w[0:P, :], l1w[P : 2 * P, :]], axis=1)  # [128, 2, 128]
    l1b = np.asarray(inputs["lin1_b"], np.float32).reshape(H // 2, 1)
    l2w = np.asarray(inputs["lin2_w"], np.float32).reshape(H // 2, 1)
    l2b = float(np.asarray(inputs["lin2_b"], np.float32).reshape(()))

    cnt = np.bincount(batch, minlength=N_GRAPHS).astype(np.float32)
    recip_g = 1.0 / np.maximum(cnt, 1.0)  # [256], per graph

    common = {
        "w1p": w1p, "w2p": w2p,
        "b1p": nn1_b.astype(BF), "b2p": b2p.astype(BF),
        "r1wb": r1wb.astype(BF), "r2wb": r2wb.astype(BF),
        "b2sbb": bias2.astype(BF),
        "l1wb": l1wb.astype(BF),
        "l1bcol": l1b.astype(np.float32),
        "l2wcol": l2w.astype(np.float32),
        "l2bcol": np.full((GW, 1), l2b, np.float32),
        "identb": np.eye(P, dtype=BF),
    }

    in_maps = []
    sel_nz_all = set()
    for c in range(NCORES):
        eids = per_core[c]
        ne = len(eids)
        srcs = src[eids]
        dstl = (dst[eids] - nr[c]).astype(np.int64)

        xg = x[srcs, :].astype(BF)  # [ne, 64]
        xsrc2 = np.zeros((P, 2, e_pad), BF)
        for s in range(2):
            iofs = (32 * (g32 + s) + j32) % DN  # [128]
            xsrc2[:, s, 0:ne] = xg[:, iofs].T

        ag = attr[eids, :]  # [ne, 32]
        bcq = np.zeros((P, 8, e_pad), BF)
        for q in range(8):
            for g in range(4):
                bcq[32 * g : 32 * g + 32, q, 0:ne] = ag[:, 4 * q + g].astype(BF)[None, :]

        scm = np.zeros((P, NSC * P), BF)
        for bi, (e, n) in enumerate(sc_blocks):
            seg = dstl[e * P : min((e + 1) * P, ne)]
            for p, dv in enumerate(seg):
                q = dv - n * P
                if 0 <= q < P:
                    scm[p, bi * P + q] = 1.0

        # pool scatter: node (local) -> graph (local window), recip weight
        batch_l = batch[nr[c] : nr[c + 1]] - cuts[c]
        gl = batch[nr[c] : nr[c + 1]]
        scp = np.zeros((P, NT * GW), BF)
        for p_loc in range(nr[c + 1] - nr[c]):
            n_t, p_p = p_loc // P, p_loc % P
            scp[p_p, n_t * GW + batch_l[p_loc]] = BF(recip_g[gl[p_loc]])

        xshT = np.ones((DN + 1, NSH), BF)
        xshT[0:DN, :] = 0.0
        xshT[0:DN, 0 : nr[c + 1] - nr[c]] = x[nr[c] : nr[c + 1], :].astype(BF).T
        xshT[DN, nr[c + 1] - nr[c] :] = 0.0

        snd_idx = np.full(S, -1, np.int64)
        for d in range(NCORES):
            rows = send_rows[c][d]
            snd_idx[d * SB : d * SB + len(rows)] = rows
        SBT = S // P
        selm = np.zeros((P, SBT * NT * P), BF)
        for row in range(S):
            v = snd_idx[row]
            if v < 0:
                continue
            r, q = row // P, row % P
            nt_, npart = int(v) // P, int(v) % P
            selm[npart, (r * NT + nt_) * P + q] = 1.0
        h1src_idx = np.zeros(e_pad, np.int16)
        for d2 in range(NCORES):
            pos, inv = recv_pos_parts[c][d2]
            h1src_idx[pos] = d2 * SB + inv
        for row in range(S):
            v = snd_idx[row]
            if v >= 0:
                sel_nz_all.add((row // P, int(v) // P))

        m = dict(common)
        m["xsrc2"] = xsrc2
        m["bcq"] = bcq
        m["scm"] = scm
        m["scp"] = scp
        m["sel"] = selm
        m["xshT"] = xshT
        m["h1src_w"] = _wrap_idx(h1src_idx, e_pad)
        in_maps.append(m)

    zb = (
        bool(np.all(np.asarray(inputs["nn1_b"]) == 0)),
        bool(np.all(np.asarray(inputs["nn2_b"]) == 0))
        and bool(np.all(np.asarray(inputs["bias2"]) == 0)),
        bool(np.all(np.asarray(inputs["lin1_b"]) == 0)),
    )
    _PREP["args"] = (e_pad, S, tuple(sc_blocks), NT, GW, zb)
    _PREP["sel_blocks"] = tuple(sorted(sel_nz_all))
    _PREP["cuts"] = cuts
    return e_pad, in_maps


def run_debug(upto, **inputs):
    e_pad, in_maps = _prep_inputs(inputs)
    ep, S, blocks, NT, GW, zb = _PREP["args"]
    nc = _build(ep, S, list(blocks), NT, GW, zb=zb, upto=upto,
                sel_blocks=_PREP["sel_blocks"])
    res = bass_utils.run_bass_kernel_spmd(nc, in_maps, core_ids=list(range(NCORES)))
    return e_pad, res


def kernel(**inputs) -> np.ndarray:
    e_pad, in_maps = _prep_inputs(inputs)
    key = _PREP["args"][:5]
    if key not in _cache:
        ep, S, blocks, NT, GW, zb = _PREP["args"]
        _cache[key] = _build(ep, S, list(blocks), NT, GW, zb=zb,
                             sel_blocks=_PREP["sel_blocks"])
        _cache[e_pad] = _cache[key]  # test.py compat (keyed by e_pad)
    nc = _cache[key]
    res = bass_utils.run_bass_kernel_spmd(nc, in_maps, core_ids=list(range(NCORES)))
    cuts = _PREP["cuts"]
    out = np.zeros((N_GRAPHS, 1), np.float32)
    for c in range(NCORES):
        w = cuts[c + 1] - cuts[c]
        out[cuts[c] : cuts[c + 1], :] = np.asarray(
            res.results[c]["out"], dtype=np.float32
        )[0:w, :]
    return out


# revision 22
# speedup vs baseline: 1.3037x; 1.0419x over previous
"""Trainium2 Bass kernel for nn_NNModel2 (2x NNConv GNN + pooled MLP readout).

Self-contained: accepts FULL inputs, shards across 8 NeuronCores, returns the
FULL [256, 1] output.

v4 design:
  - Graph-aligned node ranges: every graph's nodes live on one core, so the
    pooled readout is fully local; each core writes its own [GW, 1] slice and
    the host concatenates (no tail collectives).
  - conv layers use the z-trick: z[e,(k,i)] = attr[e,k]*x[src,i]; msg = z @ W'
    as PSUM-accumulated matmuls over 128-row (k,i) blocks.
  - Hybrid replication: cheap (low in-degree) remote source nodes are
    replicated locally so the first L conv2 edge-tiles are fully local-src.
    Those tiles (z-mults + matmuls + scatter) run DURING the AllToAll that
    fetches the remaining h1 rows, hiding most of the collective.
  - h1 exchange: AllToAll of deduped per-(src-owner, dst-owner) rows, then a
    dma_gather (transpose) for the remote edge columns plus partition-rotated
    copies for the conv2 s=1..3 blocks (rotations run on Pool/ACT, hidden
    under conv2 compute).
"""

import sys

sys.path.insert(0, "/opt/trn_rl_repo")

import numpy as np
import ml_dtypes

from concourse import bacc, bass, mybir
import concourse.tile as tile
from concourse import bass_utils

P = 128
NCORES = 8
N_NODES = 4096
N_EDGES = 8192
N_GRAPHS = 256
DN = 64
DE = 32
H = 256
L_LOC = 2  # conv2 edge-tiles made fully local via replication

F32 = mybir.dt.float32
BF16 = mybir.dt.bfloat16
I16 = mybir.dt.int16
AF = mybir.ActivationFunctionType
ALU = mybir.AluOpType
BF = ml_dtypes.bfloat16

_cache = {}
_PREP = {}


def _wrap_idx(idx, n):
    idx = np.asarray(idx, dtype=np.int16)
    assert idx.shape == (n,) and n % 16 == 0
    return np.tile(idx.reshape(n // 16, 16).T, (8, 1)).copy()


def _build(cfg, upto="full"):
    e_pad1 = cfg["e_pad1"]  # conv1 edge array size (includes replica in-edges)
    e_pad2 = cfg["e_pad2"]  # conv2 edge count padded
    S = cfg["S"]
    NT = cfg["NT"]
    GW = cfg["GW"]
    L = cfg["L"]
    sc1 = cfg["sc1"]  # [(col, e, n)] conv1 scatter blocks
    sc2 = cfg["sc2"]  # [(col, e, n)] conv2 scatter blocks
    NSC = cfg["nsc"]  # total scm column blocks
    sel_nz = set(cfg["sel_blocks"])
    zb1, zb2, _ = cfg["zb"]

    ET1 = e_pad1 // P
    ET2 = e_pad2 // P
    EL = L * P  # local columns
    EPR = e_pad2 - EL  # remote columns
    SBT = S // P
    NSH = NT * P
    nc = bacc.Bacc(num_devices=NCORES)

    # ---- per-core inputs (host-prepped)
    xsrc2 = nc.dram_tensor("xsrc2", [P, 2, e_pad1], BF16, kind="ExternalInput")
    bcq = nc.dram_tensor("bcq", [P, 8, e_pad1], BF16, kind="ExternalInput")
    scm = nc.dram_tensor("scm", [P, NSC * P], BF16, kind="ExternalInput")
    scp = nc.dram_tensor("scp", [P, NT * GW], BF16, kind="ExternalInput")
    sel = nc.dram_tensor("sel", [P, SBT * NT * P], BF16, kind="ExternalInput")
    xshT = nc.dram_tensor("xshT", [DN + 1, NSH], BF16, kind="ExternalInput")
    h1src_w = nc.dram_tensor("h1src_w", [P, EPR // 16], I16, kind="ExternalInput")
    h1loc_w = nc.dram_tensor("h1loc_w", [P, EL // 16], I16, kind="ExternalInput")
    identb = nc.dram_tensor("identb", [P, P], BF16, kind="ExternalInput")
    # ---- shared weights (host-permuted, bf16)
    w1p = nc.dram_tensor("w1p", [P, 16, H], BF16, kind="ExternalInput")
    w2p = nc.dram_tensor("w2p", [P, 64, H], BF16, kind="ExternalInput")
    b1p = nc.dram_tensor("b1p", [DN, H], BF16, kind="ExternalInput")
    b2p = nc.dram_tensor("b2p", [P, 2, H], BF16, kind="ExternalInput")
    r1wb = nc.dram_tensor("r1wb", [DN + 1, H], BF16, kind="ExternalInput")
    r2wb = nc.dram_tensor("r2wb", [P, 2, H], BF16, kind="ExternalInput")
    b2sbb = nc.dram_tensor("b2sbb", [1, H], BF16, kind="ExternalInput")
    l1wb = nc.dram_tensor("l1wb", [P, 2, H // 2], BF16, kind="ExternalInput")
    l1bcol = nc.dram_tensor("l1bcol", [H // 2, 1], F32, kind="ExternalInput")
    l2wcol = nc.dram_tensor("l2wcol", [H // 2, 1], F32, kind="ExternalInput")
    l2bcol = nc.dram_tensor("l2bcol", [GW, 1], F32, kind="ExternalInput")
    out = nc.dram_tensor("out", [GW, 1], F32, kind="ExternalOutput")

    rg = [list(range(NCORES))]
    NAT = (NT + 1) // 2  # agg psum tiles

    # first bank-touch for conv1 agg scatter (bank = n // 2), scatter-first
    first_touch = {}
    for ci, e, n in sc1:
        first_touch.setdefault(n // 2, ("sc", ci))
    for n in range(NT):
        first_touch.setdefault(n // 2, ("root", n))

    with tile.TileContext(nc, num_cores=NCORES) as tc:
        with (
            tc.tile_pool(name="const", bufs=1) as cp,
            tc.tile_pool(name="work", bufs=3) as wp,
            tc.tile_pool(name="dram", bufs=1, space="DRAM") as dr,
        ):
            # ======== stage A: loads (SP queue), conv1-critical first.
            xsrc2_sb = cp.tile([P, 2, e_pad1], BF16)
            nc.sync.dma_start(out=xsrc2_sb[:, 0:1, :], in_=xsrc2[:, 0:1, :])
            bcq_sb = cp.tile([P, 8, e_pad1], BF16)
            nc.sync.dma_start(out=bcq_sb[:, 0:1, :], in_=bcq[:, 0:1, :])
            w1p_sb = cp.tile([P, 16, H], BF16)
            nc.sync.dma_start(out=w1p_sb[:, 0:4, :], in_=w1p[:, 0:4, :])
            nc.sync.dma_start(out=xsrc2_sb[:, 1:2, :], in_=xsrc2[:, 1:2, :])
            nc.sync.dma_start(out=bcq_sb[:, 1:2, :], in_=bcq[:, 1:2, :])
            b1p_sb = cp.tile([DN, H], BF16)
            nc.sync.dma_start(out=b1p_sb[:], in_=b1p[:])
            for c in range(1, 4):
                nc.sync.dma_start(
                    out=bcq_sb[:, 2 * c : 2 * c + 2, :], in_=bcq[:, 2 * c : 2 * c + 2, :]
                )
                if c == 1:
                    nc.sync.dma_start(out=w1p_sb[:, 4:8, :], in_=w1p[:, 4:8, :])
                if c == 2:
                    nc.sync.dma_start(out=w1p_sb[:, 8:16, :], in_=w1p[:, 8:16, :])
            scm_sb = cp.tile([P, NSC * P], BF16)
            nc.sync.dma_start(out=scm_sb[:], in_=scm[:])
            xshT_sb = cp.tile([DN + 1, NSH], BF16)
            nc.sync.dma_start(out=xshT_sb[:], in_=xshT[:])
            r1wb_sb = cp.tile([DN + 1, H], BF16)
            nc.sync.dma_start(out=r1wb_sb[:], in_=r1wb[:])
            sel_sb = cp.tile([P, SBT * NT * P], BF16)
            nc.sync.dma_start(out=sel_sb[:], in_=sel[:])
            h1src_sb = cp.tile([P, EPR // 16], I16)
            nc.sync.dma_start(out=h1src_sb[:], in_=h1src_w[:])
            h1loc_sb = cp.tile([P, EL // 16], I16)
            nc.sync.dma_start(out=h1loc_sb[:], in_=h1loc_w[:])
            ident_sb = cp.tile([P, P], BF16)
            nc.sync.dma_start(out=ident_sb[:], in_=identb[:])
            a2a_in = dr.tile([S, H], BF16)
            b2p_sb = cp.tile([P, 2, H], BF16)
            nc.sync.dma_start(out=b2p_sb[:], in_=b2p[:])
            r2wb_sb = cp.tile([P, 2, H], BF16)
            nc.sync.dma_start(out=r2wb_sb[:], in_=r2wb[:])
            b2sbb_sb = cp.tile([1, H], BF16)
            nc.sync.dma_start(out=b2sbb_sb[:], in_=b2sbb[:])
            scp_sb = cp.tile([P, NT * GW], BF16)
            nc.sync.dma_start(out=scp_sb[:], in_=scp[:])
            l1wb_sb = cp.tile([P, 2, H // 2], BF16)
            nc.sync.dma_start(out=l1wb_sb[:], in_=l1wb[:])
            l1b_sb = cp.tile([H // 2, 1], F32)
            nc.sync.dma_start(out=l1b_sb[:], in_=l1bcol[:])
            l2w_sb = cp.tile([H // 2, 1], F32)
            nc.sync.dma_start(out=l2w_sb[:], in_=l2wcol[:])
            l2b_sb = cp.tile([GW, 1], F32)
            nc.sync.dma_start(out=l2b_sb[:], in_=l2bcol[:])
            w2p_sb = cp.tile([P, 64, H], BF16)
            for c in range(4):
                nc.sync.dma_start(
                    out=w2p_sb[:, 16 * c : 16 * c + 16, :],
                    in_=w2p[:, 16 * c : 16 * c + 16, :],
                )

            with tc.tile_pool(name="psA", bufs=1, space="PSUM") as psA:
                # ======== conv1
                msg_ps = [
                    psA.tile([P, 2 * H], F32, space="PSUM", tag=f"msg{j}", name=f"msg1_{j}")
                    for j in range((ET1 + 1) // 2)
                ]

                def m1(e):
                    return msg_ps[e // 2][:, (e % 2) * H : (e % 2) * H + H]

                msbs = []
                zts1 = []
                for t in range(16):
                    q1, s1 = t // 2, t % 2
                    zt = wp.tile([P, e_pad1], BF16, tag=f"zt1_{t}", name=f"zt1_{t}", bufs=1)
                    nc.vector.tensor_tensor(
                        out=zt[:], in0=xsrc2_sb[:, s1, :], in1=bcq_sb[:, q1, :],
                        op=ALU.mult,
                    )
                    zts1.append(zt)
                T1 = 4
                if not zb1:
                    for e in range(ET1):
                        nc.tensor.matmul(
                            m1(e), lhsT=xsrc2_sb[0:DN, 0, P * e : P * (e + 1)],
                            rhs=b1p_sb[:], start=(e % 2 == 0), stop=False,
                            skip_group_check=True,
                        )
                for t in range(T1):
                    for e in range(ET1):
                        nc.tensor.matmul(
                            m1(e), lhsT=zts1[t][:, P * e : P * (e + 1)],
                            rhs=w1p_sb[:, t, :],
                            start=(zb1 and t == 0 and e % 2 == 0), stop=False,
                            skip_group_check=True,
                        )
                for e in range(ET1):
                    for t in range(T1, 16):
                        nc.tensor.matmul(
                            m1(e), lhsT=zts1[t][:, P * e : P * (e + 1)],
                            rhs=w1p_sb[:, t, :],
                            start=False, stop=(t == 15),
                            skip_group_check=True,
                        )
                    if e % 2 == 1 or e == ET1 - 1:
                        j = e // 2
                        w = min(2 * H, (ET1 - 2 * j) * H)
                        msb = wp.tile([P, 2 * H], BF16, tag="msb", bufs=5, name=f"msb1_{j}")
                        nc.scalar.activation(
                            out=msb[:, 0:w], in_=msg_ps[j][:, 0:w], func=AF.Copy
                        )
                        msbs.append(msb)

                agg_ps = [
                    psA.tile([P, 2 * H], F32, space="PSUM", tag=f"agg{j}", name=f"agg1_{j}")
                    for j in range(NAT)
                ]

                def a1(n):
                    return agg_ps[n // 2][:, (n % 2) * H : (n % 2) * H + H]

                ones_sb = cp.tile([1, P], BF16)
                nc.vector.memset(ones_sb[:], 1.0)

                for ci, e, n in sc1:
                    nc.tensor.matmul(
                        a1(n), lhsT=scm_sb[:, P * ci : P * (ci + 1)],
                        rhs=msbs[e // 2][:, (e % 2) * H : (e % 2) * H + H],
                        start=(first_touch[n // 2] == ("sc", ci)), stop=False,
                        skip_group_check=True,
                    )
                for n in range(NT):
                    nc.tensor.matmul(
                        a1(n), lhsT=xshT_sb[:, P * n : P * (n + 1)], rhs=r1wb_sb[:],
                        start=(first_touch[n // 2] == ("root", n)),
                        stop=True, skip_group_check=True,
                    )

                h1sb = cp.tile([P, NT, H], BF16)
                for n in range(NT):
                    nc.scalar.activation(
                        out=h1sb[:, n, :], in_=a1(n), func=AF.Relu,
                    )

                if upto == "h1":
                    dh = nc.dram_tensor("d_h1", [P, NT * H], F32, kind="ExternalOutput")
                    tmp = wp.tile([P, NT, H], F32, tag="dbgf")
                    nc.vector.tensor_copy(out=tmp[:], in_=h1sb[:])
                    nc.sync.dma_start(
                        out=dh[:].rearrange("p (t o) -> p t o", o=H), in_=tmp[:]
                    )

                # write h1 (incl. replica slots) to DRAM for the local gather
                h1_dram = dr.tile([NSH, H], BF16)
                nc.sync.dma_start(
                    out=h1_dram[:].rearrange("(t p) o -> p t o", p=P), in_=h1sb[:]
                )

                # ======== exchange: sendbuf rows via one-hot matmuls -> AllToAll
                snd_ps = [
                    psA.tile([P, 2 * H], F32, space="PSUM", tag=f"msg{j}", name=f"snd_{j}")
                    for j in range((SBT + 1) // 2)
                ]

                def sb_ps(r):
                    return snd_ps[r // 2][:, (r % 2) * H : (r % 2) * H + H]

                sendbuf = cp.tile([P, 2 * ((SBT + 1) // 2), H], BF16)
                for r in range(SBT):
                    rn = sorted(n for (rr, n) in sel_nz if rr == r) or [0]
                    for n in rn:
                        blk = r * NT + n
                        nc.tensor.matmul(
                            sb_ps(r), lhsT=sel_sb[:, P * blk : P * (blk + 1)],
                            rhs=h1sb[:, n, :], start=(n == rn[0] and r % 2 == 0),
                            stop=(n == rn[-1]), skip_group_check=True,
                        )
                    if r % 2 == 1 or r == SBT - 1:
                        j = r // 2
                        hi = 2 if (SBT - 2 * j) >= 2 else 1
                        nc.scalar.activation(
                            out=sendbuf[:, 2 * j : 2 * j + hi, :],
                            in_=snd_ps[j][:, 0 : hi * H], func=AF.Copy,
                        )
                        nc.gpsimd.dma_start(
                            out=a2a_in[:].rearrange("(b p) e -> p b e", p=P)[
                                :, 2 * j : 2 * j + hi, :
                            ],
                            in_=sendbuf[:, 2 * j : 2 * j + hi, :],
                        )
                a2a_out = dr.tile([S, H], BF16)
                nc.gpsimd.collective_compute(
                    "AllToAll", ALU.bypass, replica_groups=rg,
                    ins=[a2a_in[:].opt()], outs=[a2a_out[:].opt()],
                )
                # local gather (runs during the A2A): h1T columns for the
                # first L conv2 e-tiles, from local h1 (incl. replicas)
                h1locT = cp.tile([P, 2, EL], BF16)
                nc.gpsimd.dma_gather(
                    out_ap=h1locT[:], in_ap=h1_dram[:], idxs_ap=h1loc_sb[:],
                    num_idxs=EL, num_idxs_reg=EL, elem_size=H,
                    transpose=True, single_packet=False,
                )

                # h1shT via PE transposes of h1sb (during the A2A)
                h1shT = cp.tile([P, 2, NSH], BF16)
                for n in range(NT):
                    for oh in range(2):
                        tsh = psA.tile(
                            [P, P], BF16, space="PSUM", tag=f"agg{(n * 2 + oh) % 2}",
                            name=f"tsh_{n}_{oh}",
                        )
                        nc.tensor.transpose(
                            out=tsh[:], in_=h1sb[:, n, P * oh : P * (oh + 1)],
                            identity=ident_sb[:],
                        )
                        nc.scalar.activation(
                            out=h1shT[:, oh, P * n : P * (n + 1)], in_=tsh[:],
                            func=AF.Copy,
                        )

                # rotated local copies for s=1..3 (Pool + ACT, during the A2A)
                def make_rots(src_t, width, tagpfx):
                    rots = [src_t]
                    for r in range(1, 4):
                        h1r = cp.tile([P, 2, width], BF16, name=f"{tagpfx}{r}")
                        k = 0
                        for c in range(2):
                            for d in range(4):
                                t = 32 * (d + r)
                                q, slot = t % P, (c if t < P else 1 - c)
                                eng = nc.gpsimd if k % 2 == 0 else nc.scalar
                                if k % 2 == 0:
                                    eng.tensor_copy(
                                        out=h1r[32 * d : 32 * d + 32, c, :],
                                        in_=src_t[q : q + 32, slot, :],
                                    )
                                else:
                                    eng.activation(
                                        out=h1r[32 * d : 32 * d + 32, c, :],
                                        in_=src_t[q : q + 32, slot, :], func=AF.Copy,
                                    )
                                k += 1
                        rots.append(h1r)
                    return rots

                h1locrots = make_rots(h1locT, EL, "h1locrot")

                # ======== root2 early (PE, during the A2A)
                agg2_ps = [
                    psA.tile([P, 2 * H], F32, space="PSUM", tag=f"agg{j}", name=f"agg2_{j}")
                    for j in range(NAT)
                ]

                def a2(n):
                    return agg2_ps[n // 2][:, (n % 2) * H : (n % 2) * H + H]

                for n in range(NT):
                    for kh in range(2):
                        nc.tensor.matmul(
                            a2(n), lhsT=h1shT[:, kh, P * n : P * (n + 1)],
                            rhs=r2wb_sb[:, kh, :],
                            start=(n % 2 == 0 and kh == 0), stop=False,
                            skip_group_check=True,
                        )
                    if not zb2:
                        nc.tensor.matmul(
                            a2(n), lhsT=ones_sb[:], rhs=b2sbb_sb[:],
                            start=False, stop=False, skip_group_check=True,
                        )

                # ======== conv2 EARLY: local e-tiles during the A2A
                msg2_ps = [
                    psA.tile([P, 2 * H], F32, space="PSUM", tag=f"msg{j}", name=f"msg2_{j}")
                    for j in range((ET2 + 1) // 2)
                ]

                def m2(e):
                    return msg2_ps[e // 2][:, (e % 2) * H : (e % 2) * H + H]

                if not zb2:
                    for e in range(L):
                        for ih in range(2):
                            nc.tensor.matmul(
                                m2(e), lhsT=h1locT[:, ih, P * e : P * (e + 1)],
                                rhs=b2p_sb[:, ih, :], start=(ih == 0 and e % 2 == 0),
                                stop=False, skip_group_check=True,
                            )
                for b in range(64):
                    s2, q2, ih = b // 16, (b % 16) // 2, b % 2
                    zt = wp.tile([P, EL], BF16, tag="ztl", bufs=4)
                    nc.vector.tensor_tensor(
                        out=zt[:], in0=h1locrots[s2][:, ih, :],
                        in1=bcq_sb[:, q2, 0:EL], op=ALU.mult,
                    )
                    for e in range(L):
                        nc.tensor.matmul(
                            m2(e), lhsT=zt[:, P * e : P * (e + 1)], rhs=w2p_sb[:, b, :],
                            start=(zb2 and b == 0 and e % 2 == 0), stop=(b == 63),
                            skip_group_check=True,
                        )

                # early msb copies + early scatter blocks (e < L)
                msbs2 = {}
                for j in range(L // 2):
                    msb = wp.tile([P, 2 * H], BF16, tag="msb", bufs=5)
                    nc.scalar.activation(out=msb[:], in_=msg2_ps[j][:], func=AF.Copy)
                    msbs2[j] = msb

                last_of_bank = {}
                for ci, e, n in sc2:
                    last_of_bank[n // 2] = ci
                for ci, e, n in sc2:
                    if e < L:
                        nc.tensor.matmul(
                            a2(n), lhsT=scm_sb[:, P * ci : P * (ci + 1)],
                            rhs=msbs2[e // 2][:, (e % 2) * H : (e % 2) * H + H],
                            start=False, stop=(last_of_bank[n // 2] == ci),
                            skip_group_check=True,
                        )

                # ======== remote gather after the A2A, then conv2 LATE
                h1srcT = cp.tile([P, 2, EPR], BF16)
                nc.gpsimd.dma_gather(
                    out_ap=h1srcT[:], in_ap=a2a_out[:], idxs_ap=h1src_sb[:],
                    num_idxs=EPR, num_idxs_reg=EPR, elem_size=H,
                    transpose=True, single_packet=False,
                )

                if upto == "h1srcT":
                    d1 = nc.dram_tensor("d_h1srcT", [P, 2 * EPR], F32, kind="ExternalOutput")
                    tmp = wp.tile([P, 2, EPR], F32, tag="dbgf")
                    nc.vector.tensor_copy(out=tmp[:], in_=h1srcT[:])
                    nc.sync.dma_start(
                        out=d1[:].rearrange("p (c e) -> p c e", c=2), in_=tmp[:]
                    )

                h1rots = make_rots(h1srcT, EPR, "h1rot")

                if not zb2:
                    for e in range(L, ET2):
                        for ih in range(2):
                            nc.tensor.matmul(
                                m2(e), lhsT=h1srcT[:, ih, P * (e - L) : P * (e - L + 1)],
                                rhs=b2p_sb[:, ih, :], start=(ih == 0 and e % 2 == 0),
                                stop=False, skip_group_check=True,
                            )
                for b in range(64):
                    s2, q2, ih = b // 16, (b % 16) // 2, b % 2
                    zt = wp.tile([P, EPR], BF16, tag="zt", bufs=4)
                    nc.vector.tensor_tensor(
                        out=zt[:], in0=h1rots[s2][:, ih, :],
                        in1=bcq_sb[:, q2, EL:e_pad2], op=ALU.mult,
                    )
                    for e in range(L, ET2):
                        nc.tensor.matmul(
                            m2(e), lhsT=zt[:, P * (e - L) : P * (e - L + 1)],
                            rhs=w2p_sb[:, b, :],
                            start=(zb2 and b == 0 and e % 2 == 0), stop=(b == 63),
                            skip_group_check=True,
                        )

                for j in range(L // 2, (ET2 + 1) // 2):
                    w = min(2 * H, (ET2 - 2 * j) * H)
                    msb = wp.tile([P, 2 * H], BF16, tag="msb", bufs=5)
                    nc.scalar.activation(out=msb[:, 0:w], in_=msg2_ps[j][:, 0:w], func=AF.Copy)
                    msbs2[j] = msb

                for ci, e, n in sc2:
                    if e >= L:
                        nc.tensor.matmul(
                            a2(n), lhsT=scm_sb[:, P * ci : P * (ci + 1)],
                            rhs=msbs2[e // 2][:, (e % 2) * H : (e % 2) * H + H],
                            start=False, stop=(last_of_bank[n // 2] == ci),
                            skip_group_check=True,
                        )

                h2sb = cp.tile([P, NT, H], BF16)
                for n in range(NT):
                    nc.scalar.activation(
                        out=h2sb[:, n, :], in_=a2(n), func=AF.Copy,
                    )

                if upto == "h2":
                    dh = nc.dram_tensor("d_h2", [P, NT * H], F32, kind="ExternalOutput")
                    tmp = wp.tile([P, NT, H], F32, tag="dbgf")
                    nc.vector.tensor_copy(out=tmp[:], in_=h2sb[:])
                    nc.sync.dma_start(
                        out=dh[:].rearrange("p (t o) -> p t o", o=H), in_=tmp[:]
                    )

                # ======== pool + readout (fully local; graphs are core-owned)
                meanT_ps = psA.tile([P, 2, GW], F32, space="PSUM", tag="agg0", name="meanT")
                for n in range(NT):
                    for oh in range(2):
                        nc.tensor.matmul(
                            meanT_ps[:, oh, :],
                            lhsT=h2sb[:, n, P * oh : P * (oh + 1)],
                            rhs=scp_sb[:, GW * n : GW * (n + 1)],
                            start=(n == 0 and oh == 0), stop=(n == NT - 1),
                            skip_group_check=True,
                        )
                meanT_sb = cp.tile([P, 2, GW], BF16)
                nc.scalar.activation(out=meanT_sb[:], in_=meanT_ps[:], func=AF.Copy)
                z1T_ps = psA.tile([P, GW], F32, space="PSUM", tag="agg1", name="z1T")
                for oh in range(2):
                    nc.tensor.matmul(
                        z1T_ps[:],
                        lhsT=l1wb_sb[:, oh, :],
                        rhs=meanT_sb[:, oh, :],
                        start=(oh == 0), stop=(oh == 1),
                        skip_group_check=True,
                    )
                z1r = cp.tile([P, GW], F32)
                nc.scalar.activation(
                    out=z1r[:], in_=z1T_ps[:], func=AF.Relu, bias=l1b_sb[:]
                )
                o_ps = psA.tile([GW, 1], F32, space="PSUM", tag="agg2", name="oput")
                nc.tensor.matmul(
                    o_ps[:], lhsT=z1r[:], rhs=l2w_sb[:],
                    start=True, stop=True, skip_group_check=True,
                )
                osb = wp.tile([GW, 1], F32, tag="t4")
                nc.scalar.activation(
                    out=osb[:], in_=o_ps[:], func=AF.Sigmoid, bias=l2b_sb[:]
                )
                nc.sync.dma_start(out=out[:], in_=osb[:])

    nc.compile()
    return nc


def _prep_inputs(inputs):
    x = np.asarray(inputs["x"], dtype=np.float32)
    ei = np.asarray(inputs["edge_index"])
    attr = np.asarray(inputs["edge_attr"], dtype=np.float32)
    batch = np.asarray(inputs["batch"]).astype(np.int64)
    src, dst = ei[0].astype(np.int64), ei[1].astype(np.int64)
    L = L_LOC
    EL = L * P

    # ---- graph-aligned node ranges
    gstart = np.searchsorted(batch, np.arange(N_GRAPHS + 1))
    cuts = [0]
    for c in range(1, NCORES):
        cuts.append(int(np.argmin(np.abs(gstart - (N_NODES // NCORES) * c))))
    cuts.append(N_GRAPHS)
    nr = np.array([int(gstart[cuts[c]]) for c in range(NCORES + 1)])
    own_cnt = [int(nr[c + 1] - nr[c]) for c in range(NCORES)]
    win = [cuts[c + 1] - cuts[c] for c in range(NCORES)]
    GW = ((max(win) + 15) // 16) * 16

    dst_owner = np.searchsorted(nr[1:], dst, side="right")
    src_owner = np.searchsorted(nr[1:], src, side="right")
    indeg = np.bincount(dst, minlength=N_NODES)

    # ---- per-core replica selection + edge ordering
    per_core2 = []  # conv2 edges, [early(local+localized) | remote], dst-sorted per group
    extras = []  # conv1-only replica in-edges
    replicas = []  # replica node lists
    for c in range(NCORES):
        eids = np.nonzero(dst_owner == c)[0]
        is_loc = src_owner[eids] == c
        loc_cnt = int(is_loc.sum())
        rem = eids[~is_loc]
        uniq, inv, cnts = np.unique(src[rem], return_inverse=True, return_counts=True)
        order = np.argsort(indeg[uniq] / cnts, kind="stable")
        R = []
        need = EL - loc_cnt
        for i in order:
            if need <= 0:
                break
            R.append(int(uniq[i]))
            need -= int(cnts[i])
        Rset = set(R)
        localized = np.array([src[e] in Rset for e in rem])
        early = np.concatenate([eids[is_loc], rem[localized]])
        late = rem[~localized]
        early = early[np.argsort(dst[early], kind="stable")]
        late = late[np.argsort(dst[late], kind="stable")]
        # early group must fill exactly EL slots; move overflow to late
        if len(early) > EL:
            late = np.concatenate([early[EL:], late])
            late = late[np.argsort(dst[late], kind="stable")]
            early = early[:EL]
        assert len(early) == EL, f"core {c}: early {len(early)} < {EL}"
        per_core2.append(np.concatenate([early, late]))
        replicas.append(sorted(Rset))
        ex = np.nonzero(np.isin(dst, list(Rset)))[0] if Rset else np.array([], np.int64)
        extras.append(ex)

    ne2_max = max(len(e) for e in per_core2)
    e_pad2 = ((ne2_max + P - 1) // P) * P
    ET2 = e_pad2 // P
    ne1_max = max(len(per_core2[c]) + len(extras[c]) for c in range(NCORES))
    e_pad1 = max(((ne1_max + P - 1) // P) * P, e_pad2)
    ET1 = e_pad1 // P
    EPR = e_pad2 - EL

    NT = (max(own_cnt[c] + len(replicas[c]) for c in range(NCORES)) + P - 1) // P
    NSH = NT * P

    # slot maps: own node n -> n - nr[c]; replica r -> own_cnt + idx
    slot_maps = []
    for c in range(NCORES):
        sm = {}
        for i, rn in enumerate(replicas[c]):
            sm[rn] = own_cnt[c] + i
        slot_maps.append(sm)

    def slot_of(c, node):
        if nr[c] <= node < nr[c + 1]:
            return int(node - nr[c])
        return slot_maps[c][int(node)]

    # ---- scatter blocks (dedup conv1/conv2 where identical)
    # conv1: all edges (conv2 order + extras appended), dst -> slot
    # conv2: only conv2 edges
    scm_cols = []  # list of (e, n) -> column data built per core later
    sc1_keys = []  # [(colidx, e, n)]
    sc2_keys = []
    col_index = {}  # (kind, e, n) -> col;  kind: 'b'=both, '1'=conv1-only, '2'=conv2-only

    # determine block structure per (e, n) across cores: a block differs
    # between conv1/conv2 only if it contains extra-edge rows.
    ex_start = [len(per_core2[c]) for c in range(NCORES)]
    blocks1 = set()
    blocks2 = set()
    for c in range(NCORES):
        alle = np.concatenate([per_core2[c], extras[c]]) if len(extras[c]) else per_core2[c]
        slots = np.array([slot_of(c, int(d)) for d in dst[alle]])
        for e in range(ET1):
            seg = slots[e * P : (e + 1) * P]
            seg2 = slots[e * P : min((e + 1) * P, ex_start[c])]
            if len(seg):
                for n in range(int(seg.min()) // P, int(seg.max()) // P + 1):
                    blocks1.add((e, n))
            if e < ET2 and len(seg2):
                for n in range(int(seg2.min()) // P, int(seg2.max()) // P + 1):
                    blocks2.add((e, n))
    # shared if conv1 block == conv2 block (no extras rows in that (e,n))
    mixed = set()
    for c in range(NCORES):
        if not len(extras[c]):
            continue
        alle = np.concatenate([per_core2[c], extras[c]])
        slots = np.array([slot_of(c, int(d)) for d in dst[alle]])
        for pos in range(ex_start[c], len(alle)):
            e, n = pos // P, int(slots[pos]) // P
            mixed.add((e, n))
    ncol = 0
    for e, n in sorted(blocks1 | blocks2):
        b1 = (e, n) in blocks1
        b2 = (e, n) in blocks2
        mx = (e, n) in mixed
        if b1 and b2 and not mx:
            col_index[("b", e, n)] = ncol
            sc1_keys.append((ncol, e, n))
            sc2_keys.append((ncol, e, n))
            ncol += 1
        else:
            if b1:
                col_index[("1", e, n)] = ncol
                sc1_keys.append((ncol, e, n))
                ncol += 1
            if b2:
                col_index[("2", e, n)] = ncol
                sc2_keys.append((ncol, e, n))
                ncol += 1
    NSC = ncol
    sc1_keys.sort(key=lambda t: (t[1], t[2]))
    sc2_keys.sort(key=lambda t: (t[1], t[2]))

    # ---- A2A send rows: only for conv2 edge positions >= EL
    send_rows = [[None] * NCORES for _ in range(NCORES)]
    recv_pos_parts = [[None] * NCORES for _ in range(NCORES)]
    maxrows = 1
    for d in range(NCORES):
        late = per_core2[d][EL:]
        srcs = src[late]
        co = src_owner[late]
        for c in range(NCORES):
            mask = co == c
            uniq, inv = np.unique(srcs[mask] - nr[c], return_inverse=True)
            send_rows[c][d] = uniq
            recv_pos_parts[d][c] = (np.nonzero(mask)[0], inv)
            maxrows = max(maxrows, len(uniq))
    SB = ((maxrows + 15) // 16) * 16
    S = ((NCORES * SB + P - 1) // P) * P
    SB = S // NCORES
    assert S % P == 0

    # host-permuted weights (shared)
    nn1_w = np.asarray(inputs["nn1_w"], np.float32)
    nn2_w = np.asarray(inputs["nn2_w"], np.float32)
    pidx = np.arange(P)
    g32 = pidx // 32
    j32 = pidx % 32
    nn1_r = nn1_w.reshape(DE, DN, H)
    w1p = np.zeros((P, 16, H), np.float32)
    for t in range(16):
        q, s = t // 2, t % 2
        k = 4 * q + g32
        i = (32 * (g32 + s) + j32) % DN
        w1p[:, t, :] = nn1_r[k, i, :]
    w1p = w1p.astype(BF)
    nn2_r = nn2_w.reshape(DE, H, H)
    w2p = np.zeros((P, 64, H), np.float32)
    for b in range(64):
        s, q, ih = b // 16, (b % 16) // 2, b % 2
        k = 4 * q + g32
        i = (128 * ih + 32 * (g32 + s) + j32) % H
        w2p[:, b, :] = nn2_r[k, i, :]
    w2p = w2p.astype(BF)

    nn1_b = np.asarray(inputs["nn1_b"], np.float32).reshape(DN, H)
    nn2_b = np.asarray(inputs["nn2_b"], np.float32).reshape(H, H)
    b2p = np.stack([nn2_b[0:P, :], nn2_b[P : 2 * P, :]], axis=1)
    r1w = np.asarray(inputs["root1_w"], np.float32)
    bias1 = np.asarray(inputs["bias1"], np.float32)
    r1wb = np.concatenate([r1w, bias1.reshape(1, H)], axis=0)
    r2w = np.asarray(inputs["root2_w"], np.float32)
    r2wb = np.stack([r2w[0:P, :], r2w[P : 2 * P, :]], axis=1)
    bias2 = np.asarray(inputs["bias2"], np.float32).reshape(1, H)
    l1w = np.asarray(inputs["lin1_w"], np.float32)
    l1wb = np.stack([l1w[0:P, :], l1w[P : 2 * P, :]], axis=1)
    l1b = np.asarray(inputs["lin1_b"], np.float32).reshape(H // 2, 1)
    l2w = np.asarray(inputs["lin2_w"], np.float32).reshape(H // 2, 1)
    l2b = float(np.asarray(inputs["lin2_b"], np.float32).reshape(()))

    cnt = np.bincount(batch, minlength=N_GRAPHS).astype(np.float32)
    recip_g = 1.0 / np.maximum(cnt, 1.0)

    common = {
        "w1p": w1p, "w2p": w2p,
        "b1p": nn1_b.astype(BF), "b2p": b2p.astype(BF),
        "r1wb": r1wb.astype(BF), "r2wb": r2wb.astype(BF),
        "b2sbb": bias2.astype(BF),
        "l1wb": l1wb.astype(BF),
        "l1bcol": l1b.astype(np.float32),
        "l2wcol": l2w.astype(np.float32),
        "l2bcol": np.full((GW, 1), l2b, np.float32),
        "identb": np.eye(P, dtype=BF),
    }

    in_maps = []
    sel_nz_all = set()
    for c in range(NCORES):
        e2 = per_core2[c]
        alle = np.concatenate([e2, extras[c]]) if len(extras[c]) else e2
        ne1 = len(alle)
        ne2 = len(e2)
        srcs = src[alle]
        slots_d = np.array([slot_of(c, int(d)) for d in dst[alle]])

        xg = x[srcs, :].astype(BF)
        xsrc2 = np.zeros((P, 2, e_pad1), BF)
        for s in range(2):
            iofs = (32 * (g32 + s) + j32) % DN
            xsrc2[:, s, 0:ne1] = xg[:, iofs].T

        ag = attr[alle, :]
        bcq = np.zeros((P, 8, e_pad1), BF)
        for q in range(8):
            for g in range(4):
                bcq[32 * g : 32 * g + 32, q, 0:ne1] = ag[:, 4 * q + g].astype(BF)[None, :]

        scm = np.zeros((P, NSC * P), BF)

        def fill_block(colidx, e, n, limit):
            seg = slots_d[e * P : min((e + 1) * P, limit)]
            for p, sv in enumerate(seg):
                q = sv - n * P
                if 0 <= q < P:
                    scm[p, colidx * P + q] = 1.0

        for (kind, e, n), ci in col_index.items():
            if kind == "b":
                fill_block(ci, e, n, ne1)  # no extras in this block; same either way
            elif kind == "1":
                fill_block(ci, e, n, ne1)
            else:
                fill_block(ci, e, n, ne2)

        batch_l = batch[nr[c] : nr[c + 1]] - cuts[c]
        gl = batch[nr[c] : nr[c + 1]]
        scp = np.zeros((P, NT * GW), BF)
        for p_loc in range(own_cnt[c]):
            n_t, p_p = p_loc // P, p_loc % P
            scp[p_p, n_t * GW + batch_l[p_loc]] = BF(recip_g[gl[p_loc]])

        xshT = np.zeros((DN + 1, NSH), BF)
        nloc = own_cnt[c] + len(replicas[c])
        xs = np.concatenate([
            x[nr[c] : nr[c + 1], :],
            x[np.array(replicas[c], np.int64), :] if replicas[c] else np.zeros((0, DN), np.float32),
        ])
        xshT[0:DN, 0:nloc] = xs.astype(BF).T
        xshT[DN, 0:nloc] = 1.0

        snd_idx = np.full(S, -1, np.int64)
        for d in range(NCORES):
            rows = send_rows[c][d]
            snd_idx[d * SB : d * SB + len(rows)] = rows
        SBT = S // P
        selm = np.zeros((P, SBT * NT * P), BF)
        for row in range(S):
            v = snd_idx[row]
            if v < 0:
                continue
            r, q = row // P, row % P
            nt_, npart = int(v) // P, int(v) % P
            selm[npart, (r * NT + nt_) * P + q] = 1.0
            sel_nz_all.add((r, nt_))
        h1src_idx = np.zeros(EPR, np.int16)
        for d2 in range(NCORES):
            pos, inv = recv_pos_parts[c][d2]
            h1src_idx[pos] = d2 * SB + inv
        h1loc_idx = np.array(
            [slot_of(c, int(s)) for s in src[e2[0:EL]]], np.int16
        )

        m = dict(common)
        m["xsrc2"] = xsrc2
        m["bcq"] = bcq
        m["scm"] = scm
        m["scp"] = scp
        m["sel"] = selm
        m["xshT"] = xshT
        m["h1src_w"] = _wrap_idx(h1src_idx, EPR)
        m["h1loc_w"] = _wrap_idx(h1loc_idx, EL)
        in_maps.append(m)

    zb = (
        bool(np.all(np.asarray(inputs["nn1_b"]) == 0)),
        bool(np.all(np.asarray(inputs["nn2_b"]) == 0))
        and bool(np.all(np.asarray(inputs["bias2"]) == 0)),
        bool(np.all(np.asarray(inputs["lin1_b"]) == 0)),
    )
    cfg = {
        "e_pad1": e_pad1, "e_pad2": e_pad2, "S": S, "NT": NT, "GW": GW, "L": L,
        "sc1": tuple(sc1_keys), "sc2": tuple(sc2_keys), "nsc": NSC,
        "sel_blocks": tuple(sorted(sel_nz_all)), "zb": zb,
    }
    _PREP["cfg"] = cfg
    _PREP["cuts"] = cuts
    return e_pad2, in_maps


def run_debug(upto, **inputs):
    e_pad, in_maps = _prep_inputs(inputs)
    nc = _build(_PREP["cfg"], upto=upto)
    res = bass_utils.run_bass_kernel_spmd(nc, in_maps, core_ids=list(range(NCORES)))
    return e_pad, res


def kernel(**inputs) -> np.ndarray:
    e_pad, in_maps = _prep_inputs(inputs)
    cfg = _PREP["cfg"]
    key = tuple(sorted((k, v) for k, v in cfg.items() if k != "zb")) + (cfg["zb"],)
    if key not in _cache:
        _cache[key] = _build(cfg)
        _cache[e_pad] = _cache[key]  # test.py compat (keyed by e_pad)
    nc = _cache[key]
    res = bass_utils.run_bass_kernel_spmd(nc, in_maps, core_ids=list(range(NCORES)))
    cuts = _PREP["cuts"]
    out = np.zeros((N_GRAPHS, 1), np.float32)
    for c in range(NCORES):
        w = cuts[c + 1] - cuts[c]
        out[cuts[c] : cuts[c + 1], :] = np.asarray(
            res.results[c]["out"], dtype=np.float32
        )[0:w, :]
    return out
